# revision 40
# baseline (speedup 1.0000x reference)
"""Trainium2 Bass kernel for nn_Loss_dict_50646254354805 (NeRF-style loss).

Self-contained: accepts FULL inputs, shards across 8 NeuronCores (rays for
the per-ray losses, samples for the hash loss), runs one SPMD Bass module,
host-sums the 8 partial scalars.

Inter-loss: the reference's blur_step_function + sorted_interp_quad is
evaluated in a merged domain. Keys are uint16 quantized values (14-bit grid)
with 2-bit source tags, bitonic-merged at 2x DVE rate; per-slot values come
from the keys (grid error ~6e-5, validated ~1e-2 rel on the inter terms,
~0.5% on the total loss vs 2e-2 budget); the +-radio slopes are scattered
as exact f32 halves (their telescoping cancellation needs full precision).
Density/CDF reconstruction runs as masked prefix scans on the Pool engine;
conversions/relu/square run on the Activation engine; counts, positions and
compaction indices are uint16 DVE ops at 2-4x rate.
"""
import numpy as np

import concourse.bass as bass
import concourse.bass_isa as bass_isa
import concourse.mybir as mybir
import concourse.tile as tile
from concourse import bacc
from concourse.bass_utils import run_bass_kernel_spmd

dt = mybir.dt
Alu = mybir.AluOpType
AX = mybir.AxisListType
ACTF = mybir.ActivationFunctionType
P = 128

# problem constants
PULSE = (0.01, 0.005)
W_RGB, W_INTER, W_DIST, W_HASH = 1.0, 1.0, 0.01, 0.1
NUM_SEGMENTS = 65536
R, N = 4096, 48
M = R * N
N_CORES = 8
RPC = R // N_CORES            # rays per core (512)
NBLK = RPC // P               # ray tiles per core (4)
MPC = M // N_CORES            # hash samples per core (24576)
HALO = 64                     # hash run halo
HROW = MPC // P               # hash samples per partition (192)
HCOLS = HROW + HALO + 1       # loaded cols per partition (257)
HSLICE = HALO + MPC + HALO    # per-core hash slice length (24704)

# key quantization: key = trunc((v + OFF) * S4), tags in the low 2 bits
S4 = 63000.0
OFF = 0.02
PADK = 0xFFFC                 # pad key (tag 0, larger than any real key)

# per-level geometry
LVL = {0: dict(X=257, n2=512), 1: dict(X=97, n2=256)}
for _L in LVL.values():
    _L["EW"] = ((_L["X"] + 98 + 1 + 7) // 8) * 8        # 360 / 200
    _L["LW"] = _L["EW"] + 24                            # 384 / 224
    _L["NL"] = NBLK * _L["LW"]                          # 1536 / 896
    _L["SL"] = NBLK * _L["n2"]                          # 2048 / 1024
    _L["NW"] = NBLK * (_L["X"] - 1)                     # 1024 / 384
    _L["QWS"] = _L["LW"] - 98                           # 286 / 126
    _L["NQ"] = NBLK * _L["QWS"]


def _ts_int(eng, out, in0, imm1, op0, imm2=None, op1=None):
    """tensor_scalar with int32 immediates (for bitwise/compare ops)."""
    ins_ = [eng.lower_ap(in0), mybir.ImmediateValue(dtype=dt.int32, value=int(imm1))]
    kw = dict(op0=op0)
    if imm2 is not None:
        ins_.append(mybir.ImmediateValue(dtype=dt.int32, value=int(imm2)))
        kw["op1"] = op1
    return eng.add_instruction(mybir.InstTensorScalarPtr(
        name=eng.bass.get_next_instruction_name(),
        ins=ins_, outs=[eng.lower_ap(out)], **kw))


def _blk(ap, n2):
    return ap.rearrange("p (b n) -> p b n", b=NBLK)


def _merge_stages(VE, bufa, bufb, width, d_list, descending=False):
    """Full bitonic merge stages (ping-pong) over [P, G*width] u16 tiles."""
    cur, nxt = bufa, bufb
    for d in d_list:
        c3 = cur[:].rearrange("p (c td) -> p c td", td=2 * d)
        n3 = nxt[:].rearrange("p (c td) -> p c td", td=2 * d)
        lo_in, hi_in = c3[:, :, 0:d], c3[:, :, d:2 * d]
        if descending:
            VE.tensor_tensor(n3[:, :, 0:d], lo_in, hi_in, Alu.max)
            VE.tensor_tensor(n3[:, :, d:2 * d], lo_in, hi_in, Alu.min)
        else:
            VE.tensor_tensor(n3[:, :, 0:d], lo_in, hi_in, Alu.min)
            VE.tensor_tensor(n3[:, :, d:2 * d], lo_in, hi_in, Alu.max)
        cur, nxt = nxt, cur
    return cur, nxt


def _emit_level_p1(nc, tc, pool, lvl, s_sh, radio, b1t, aps, accs):
    """Phase 1: merge, flags/counts, scatters, exact values, radio."""
    VE, PL, ACT, SP = nc.vector, nc.gpsimd, nc.scalar, nc.sync
    L = LVL[lvl]
    X, n2, EW, LW, NL, SL, NW, QWS, NQ = (L["X"], L["n2"], L["EW"], L["LW"],
                                          L["NL"], L["SL"], L["NW"], L["QWS"],
                                          L["NQ"])
    pw = PULSE[lvl]

    def blkL(ap):
        return ap.rearrange("p (b n) -> p b n", b=NBLK)

    st = dict(blkL=blkL)

    # ---------- per-level constants (one batched DMA on SP) ----------
    iotas = pool.tile([P, NL + NBLK * X], dt.int16, tag="iotas")
    SP.dma_start(iotas[:], aps[f"c_iota_l{lvl}"][:, 0:NL + NBLK * X])
    iotaC = iotas[:][:, 0:NL]
    aps_iotaxl = iotas[:][:, NL:]
    maskf = pool.tile([P, NL], dt.float32, tag="maskf")
    PL.memset(maskf[:], 1.0)
    PL.memset(blkL(maskf[:])[:, :, 0:1], 0.0)
    st["maskf"] = maskf

    # ---------- inputs (one batched DMA: [ps | pw] per ray) ----------
    pspw = pool.tile([P, NBLK * (2 * X - 1)], dt.float32, tag="pspw")
    SP.dma_start(_blk(pspw[:], 2 * X - 1),
                 aps[f"pspw{lvl}"].rearrange("(b p) x -> p b x", p=P))
    xt3 = _blk(pspw[:], 2 * X - 1)[:, :, 0:X]
    pwt3 = _blk(pspw[:], 2 * X - 1)[:, :, X:2 * X - 1]
    st["pwt3"] = pwt3
    dinv = pool.tile([P, NW], dt.float32, tag="dinv")
    ACT.activation(_blk(dinv[:], X - 1), pwt3, ACTF.Copy, bias=1e-5)
    VE.reciprocal(dinv[:], dinv[:])
    st["dinv"] = dinv

    # ---------- big merge: queries + events (from b1t), ascending ----------
    B0 = pool.tile([P, SL], dt.uint16, tag="big0")
    B1 = pool.tile([P, SL], dt.uint16, tag="big1")
    b03 = _blk(B0[:], n2)
    PL.memset(b03[:, :, X:n2 - 128], PADK)
    # quantized query keys written straight into the merge buffer
    ACT.activation(b03[:, :, 0:X], xt3, ACTF.Copy, scale=S4, bias=OFF * S4)
    _ts_int(VE, b03[:, :, 0:X], b03[:, :, 0:X], 0xFFFC, Alu.bitwise_and)
    b1f = b1t[:].rearrange("p (g n) -> p g n", n=128)
    g0 = lvl * NBLK
    VE.tensor_copy(b03[:, :, n2 - 128:n2], b1f[:, g0:g0 + NBLK, ::-1])
    # first stage: only the trailing 98 pairs touch real data
    d0 = n2 // 2
    VE.tensor_tensor(_blk(B1[:], n2)[:, :, d0 - 98:d0],
                     b03[:, :, d0 - 98:d0], b03[:, :, n2 - 98:n2], Alu.min)
    VE.tensor_tensor(_blk(B1[:], n2)[:, :, n2 - 98:n2],
                     b03[:, :, d0 - 98:d0], b03[:, :, n2 - 98:n2], Alu.max)
    VE.tensor_copy(_blk(B1[:], n2)[:, :, 0:d0 - 98], b03[:, :, 0:d0 - 98])
    VE.tensor_copy(_blk(B1[:], n2)[:, :, d0:n2 - 98], b03[:, :, d0:n2 - 98])
    ds_rest = [n2 // 4]
    while ds_rest[-1] > 1:
        ds_rest.append(ds_rest[-1] // 2)
    Kt, Ksc = _merge_stages(VE, B1, B0, n2, ds_rest)
    mS = _blk(Kt[:], n2)[:, :, 0:LW]       # merged keys, strided [P,NBLK,LW]
    st["mS"] = mS
    st["Ksc"] = Ksc

    # ---------- flags / counts (u16) ----------
    ev16 = pool.tile([P, NL], dt.uint16, tag="ev16")
    _ts_int(VE, blkL(ev16[:]), mS, 1, Alu.bitwise_and)
    em16 = pool.tile([P, NL], dt.uint16, tag="em16")
    _ts_int(VE, blkL(em16[:]), mS, 3, Alu.bitwise_and, 1, Alu.is_equal)
    ep16 = pool.tile([P, NL], dt.uint16, tag="ep16")
    _ts_int(VE, blkL(ep16[:]), mS, 3, Alu.bitwise_and, 3, Alu.is_equal)
    C16 = pool.tile([P, NL], dt.uint16, tag="C16")
    PL.tensor_tensor_scan(C16[:], maskf[:], ev16[:], 0.0, Alu.mult, Alu.add)
    Cm16 = pool.tile([P, NL], dt.uint16, tag="Cm16")
    PL.tensor_tensor_scan(Cm16[:], maskf[:], em16[:], 0.0, Alu.mult, Alu.add)
    st["ev16"] = ev16
    st["ep16"] = ep16

    # ---------- event position scatters ----------
    tmp16 = pool.tile([P, NL], dt.uint16, tag="tmp16")
    idx16 = pool.tile([P, NL], dt.int16, tag="idx16")
    t3 = blkL(tmp16[:])
    i3 = blkL(idx16[:])
    C3, Cm3, em3, ep3 = (blkL(C16[:]), blkL(Cm16[:]), blkL(em16[:]),
                         blkL(ep16[:]))
    pos_m = pool.tile([P, NBLK * 64], dt.uint16, tag="pos_m")
    pos_p = pool.tile([P, NBLK * 64], dt.uint16, tag="pos_p")
    for which, pos in ((0, pos_m), (1, pos_p)):
        if which == 0:
            VE.tensor_tensor(t3[:, :, 0:EW], Cm3[:, :, 0:EW], em3[:, :, 0:EW],
                             Alu.mult)
        else:
            VE.tensor_tensor(t3[:, :, 0:EW], C3[:, :, 0:EW], Cm3[:, :, 0:EW],
                             Alu.subtract)
            VE.tensor_tensor(t3[:, :, 0:EW], t3[:, :, 0:EW], ep3[:, :, 0:EW],
                             Alu.mult)
        _ts_int(VE, i3[:, :, 0:EW], t3[:, :, 0:EW], -1, Alu.add)
        for b in range(NBLK):
            PL.local_scatter(pos[:, b * 64:(b + 1) * 64],
                             iotaC[:, b * LW:b * LW + EW].bitcast(dt.uint16),
                             idx16[:, b * LW:b * LW + EW], channels=P,
                             num_elems=64, num_idxs=EW)

    # ---------- radio scatter (exact f32 halves) ----------
    CW = NBLK * 128 + NBLK * X
    idxcat = pool.tile([P, CW], dt.int16, tag="idxcat")
    tgt16 = idxcat[:][:, 0:NBLK * 128]
    tg3 = _blk(tgt16, 128)
    pm3 = _blk(pos_m[:], 64)
    pp3 = _blk(pos_p[:], 64)
    for b in range(NBLK):
        # iotaC data carries +b*QWS; fold its removal into the block offset
        _ts_int(VE, tg3[:, b, 0:49], pm3[:, b, 0:49], b * (LW - QWS) - 1, Alu.add)
        _ts_int(VE, tg3[:, b, 49:98], pp3[:, b, 0:49], b * (LW - QWS) - 1, Alu.add)
    PL.memset(tg3[:, :, 98:128], -1)

    radcat = pool.tile([P, NBLK * 128], dt.float32, tag="radcat")
    r3 = _blk(radcat[:], 128)
    VE.tensor_copy(r3[:, :, 0:49], _blk(radio[:], 49))
    VE.tensor_scalar(r3[:, :, 49:98], _blk(radio[:], 49), -1.0, None, Alu.mult)
    PL.memset(r3[:, :, 98:128], 0.0)
    rc_u = radcat[:].bitcast(dt.uint16).rearrange("p (n two) -> p n two", two=2)
    rad_lo = pool.tile([P, NBLK * 128], dt.uint16, tag="rad_lo")
    rad_hi = pool.tile([P, NBLK * 128], dt.uint16, tag="rad_hi")
    VE.tensor_copy(rad_lo[:], rc_u[:, :, 0])
    VE.tensor_copy(rad_hi[:], rc_u[:, :, 1])
    rl_t = pool.tile([P, NL], dt.uint16, tag="rl_t")
    rh_t = pool.tile([P, NL], dt.uint16, tag="rh_t")
    PL.local_scatter(rl_t[:], rad_lo[:], tgt16, channels=P,
                     num_elems=NL, num_idxs=NBLK * 128)
    PL.local_scatter(rh_t[:], rad_hi[:], tgt16, channels=P,
                     num_elems=NL, num_idxs=NBLK * 128)
    radio_m = pool.tile([P, NL], dt.float32, tag="radio_m")
    rm_u = radio_m[:].bitcast(dt.uint16).rearrange("p (n two) -> p n two", two=2)
    ACT.activation(rm_u[:, :, 0], rl_t[:], ACTF.Copy)
    ACT.activation(rm_u[:, :, 1], rh_t[:], ACTF.Copy)
    st["radio_m"] = radio_m

    # ---------- compaction indices (reused later for the cdf compact) ------
    qf16 = em16                           # em16 dead after pos idx
    _ts_int(VE, blkL(qf16[:]), mS, 3, Alu.bitwise_and, 0, Alu.is_equal)
    tq = tmp16                            # tmp16 dead after pos idx
    VE.tensor_tensor(tq[:], iotaC.bitcast(dt.uint16), C16[:], Alu.subtract)
    VE.tensor_tensor(tq[:], tq[:], qf16[:], Alu.mult)
    idxq = pool.tile([P, NL], dt.int16, tag="idxq")
    _ts_int(VE, idxq[:], tq[:], -1, Alu.add)
    st["idxq"] = idxq

    # ---------- exact per-slot values (queries + events, one scatter) ------
    i0q = qf16                            # qf16 dead after idxq
    PL.local_scatter(i0q[:, 0:NQ], C16[:], idxq[:], channels=P,
                     num_elems=NQ, num_idxs=NL)
    VE.tensor_tensor(_blk(idxcat[:][:, NBLK * 128:CW], X).bitcast(dt.uint16),
                     aps_iotaxl.bitcast(dt.uint16).rearrange(
                         "p (b n) -> p b n", b=NBLK),
                     _blk(i0q[:, 0:NQ], QWS)[:, :, 0:X], Alu.add)
    emsh = pool.tile([P, NBLK * 49], dt.float32, tag="emsh")
    ACT.activation(_blk(emsh[:], 49), s_sh, ACTF.Copy, bias=-pw)
    epsh = pool.tile([P, NBLK * 49], dt.float32, tag="epsh")
    ACT.activation(_blk(epsh[:], 49), s_sh, ACTF.Copy, bias=pw)
    vc_lo = pool.tile([P, CW], dt.uint16, tag="vc_lo")
    vc_hi = pool.tile([P, CW], dt.uint16, tag="vc_hi")
    em_u = emsh[:].bitcast(dt.uint16).rearrange("p (b n two) -> p b n two",
                                                b=NBLK, two=2)
    ep_u = epsh[:].bitcast(dt.uint16).rearrange("p (b n two) -> p b n two",
                                                b=NBLK, two=2)
    for half, vc, hname in ((0, vc_lo, "pslo"), (1, vc_hi, "pshi")):
        vch = _blk(vc[:][:, 0:NBLK * 128], 128)
        VE.tensor_copy(vch[:, :, 0:49], em_u[:, :, :, half])
        VE.tensor_copy(vch[:, :, 49:98], ep_u[:, :, :, half])
        PL.memset(vch[:, :, 98:128], 0)
        SP.dma_start(_blk(vc[:][:, NBLK * 128:CW], X),
                     aps[f"{hname}{lvl}"].rearrange("(b p) x -> p b x", p=P))
    vl_t = pool.tile([P, NL], dt.uint16, tag="vl_t")
    vh_t = pool.tile([P, NL], dt.uint16, tag="vh_t")
    PL.local_scatter(vl_t[:], vc_lo[:], idxcat[:], channels=P,
                     num_elems=NL, num_idxs=CW)
    PL.local_scatter(vh_t[:], vc_hi[:], idxcat[:], channels=P,
                     num_elems=NL, num_idxs=CW)
    v = pool.tile([P, NL], dt.float32, tag="v")
    v_u = v[:].bitcast(dt.uint16).rearrange("p (n two) -> p n two", two=2)
    ACT.activation(v_u[:, :, 0], vl_t[:], ACTF.Copy)
    ACT.activation(v_u[:, :, 1], vh_t[:], ACTF.Copy)
    dv = pool.tile([P, NL], dt.float32, tag="dv")
    dv3 = blkL(dv[:])
    v3 = blkL(v[:])
    VE.tensor_tensor(dv3[:, :, 1:EW], v3[:, :, 1:EW], v3[:, :, 0:EW - 1],
                     Alu.subtract)
    st["dv"] = dv
    st["v"] = v
    st["vl_t"] = vl_t
    st["vh_t"] = vh_t
    return st


def _emit_level_p2(nc, tc, pool, lvl, st, aps, accs):
    """Phase 2: density chain, cdf compaction, loss tail."""
    VE, PL, ACT, SP = nc.vector, nc.gpsimd, nc.scalar, nc.sync
    L = LVL[lvl]
    X, EW, LW, NL, NW, QWS, NQ = (L["X"], L["EW"], L["LW"], L["NL"], L["NW"],
                                  L["QWS"], L["NQ"])
    blkL = st["blkL"]
    maskf, radio_m, dv, dinv = st["maskf"], st["radio_m"], st["dv"], st["dinv"]
    idxq, Ksc = st["idxq"], st["Ksc"]
    dv3 = blkL(dv[:])

    # ---------- density chain (g on Pool; w/cdf scans on DVE: the tail
    # window has DVE headroom and DVE scans are cheaper) ----------
    g = pool.tile([P, NL], dt.float32, tag="g")
    PL.tensor_tensor_scan(g[:], maskf[:], radio_m[:], 0.0, Alu.mult, Alu.add)
    wg = radio_m                          # radio_m dead after g scan
    wg3 = blkL(wg[:])
    PL.memset(wg3[:, :, 0:1], 0.0)
    PL.memset(wg3[:, :, EW:LW], 0.0)
    VE.tensor_tensor(wg3[:, :, 1:EW], dv3[:, :, 1:EW], blkL(g[:])[:, :, 0:EW - 1],
                     Alu.mult)
    w = pool.tile([P, NL], dt.float32, tag="w")
    VE.tensor_tensor_scan(w[:], maskf[:], wg[:], 0.0, Alu.mult, Alu.add)
    wc = w                                # relu in place (DVE, no hop)
    VE.tensor_scalar(wc[:], w[:], 0.0, None, Alu.max)
    scr = g                               # g dead after wg
    scr3 = blkL(scr[:])
    wc3 = blkL(wc[:])
    VE.tensor_tensor(scr3[:, :, 1:EW], wc3[:, :, 1:EW], wc3[:, :, 0:EW - 1],
                     Alu.add)
    area = wg                             # wg dead after w scan
    a3 = blkL(area[:])
    PL.memset(a3[:, :, 0:1], 0.0)
    PL.memset(a3[:, :, EW:LW], 0.0)
    # 0.5 of the trapezoid is pre-folded into the radio scale (1/(4*pw))
    VE.tensor_tensor(a3[:, :, 1:EW], scr3[:, :, 1:EW], dv3[:, :, 1:EW], Alu.mult)
    cdf = dv                              # dv dead after area
    VE.tensor_tensor_scan(cdf[:], maskf[:], area[:], 0.0, Alu.mult, Alu.add)

    # ---------- compact cdf at query slots ----------
    cdf_lo = st["ev16"]                   # dead after C scan
    cdf_hi = st["ep16"]                   # dead after pos idx
    cdf_u = cdf[:].bitcast(dt.uint16).rearrange("p (n two) -> p n two", two=2)
    VE.tensor_copy(cdf_lo[:], cdf_u[:, :, 0])
    ACT.activation(cdf_hi[:], cdf_u[:, :, 1], ACTF.Copy)
    cq_lo = st["vl_t"]                    # dead after v recombine
    cq_hi = st["vh_t"]
    PL.local_scatter(cq_lo[:, 0:NQ], cdf_lo[:], idxq[:], channels=P,
                     num_elems=NQ, num_idxs=NL)
    PL.local_scatter(cq_hi[:, 0:NQ], cdf_hi[:], idxq[:], channels=P,
                     num_elems=NQ, num_idxs=NL)
    cdfq = pool.tile([P, NBLK * X], dt.float32, tag="cdfq")
    cq_u = cdfq[:].bitcast(dt.uint16).rearrange("p (b n two) -> p b n two",
                                                b=NBLK, two=2)
    VE.tensor_copy(cq_u[:, :, 0:X, 0], _blk(cq_lo[:, 0:NQ], QWS)[:, :, 0:X])
    ACT.activation(cq_u[:, :, 0:X, 1], _blk(cq_hi[:, 0:NQ], QWS)[:, :, 0:X],
                   ACTF.Copy)

    # ---------- loss tail (all DVE: shortest cross-engine chain) ----------
    ws = cdf[:][:, 0:NW]                  # cdf dead after split
    cqf = _blk(cdfq[:], X)
    ws3 = ws.rearrange("p (b n) -> p b n", b=NBLK)
    VE.tensor_tensor(ws3, cqf[:, :, 1:X], cqf[:, :, 0:X - 1], Alu.subtract)
    t = wc[:][:, 0:NW]                    # wc dead after scr
    VE.tensor_tensor(t.rearrange("p (b n) -> p b n", b=NBLK), ws3, st["pwt3"],
                     Alu.subtract)
    r = scr[:][:, 0:NW]                   # scr dead after area
    ACT.activation(r, t, ACTF.Relu)       # concurrent with u on DVE
    u = area[:][:, 0:NW]                  # area dead after cdf scan
    VE.tensor_tensor(u, t, dinv[:], Alu.mult)
    ttro = Ksc[:].bitcast(dt.float32)[:, 0:NW]   # merge scratch, long dead
    VE.tensor_tensor_reduce(ttro, u, r, 1.0 / (R * (X - 1)), 0.0,
                            Alu.mult, Alu.add,
                            accs["inter" if lvl == 0 else "inter1"][:])


def build_module():
    nc = bacc.Bacc("TRN2", target_bir_lowering=False, debug=False,
                   enable_asserts=False, num_devices=N_CORES)
    aps = {}

    def din(name, shape, dtype=dt.float32):
        aps[name] = nc.dram_tensor(name, shape, dtype, kind="ExternalInput").ap()
    din("pdgt", [RPC, 6])
    din("sdrw", [RPC, 97])
    din("pspw0", [RPC, 513]); din("pspw1", [RPC, 193])
    din("pslo0", [RPC, 257], dt.uint16); din("pshi0", [RPC, 257], dt.uint16)
    din("pslo1", [RPC, 97], dt.uint16); din("pshi1", [RPC, 97], dt.uint16)
    din("hi0", [HSLICE], dt.uint16); din("he0", [HSLICE * 2])
    din("hi1", [HSLICE], dt.uint16); din("he1", [HSLICE * 2])
    for lvl in (0, 1):
        nl = LVL[lvl]["NL"]
        din(f"c_iota_l{lvl}", [P, nl + NBLK * LVL[lvl]["X"]], dt.int16)
    out_ap = nc.dram_tensor("out", [1, 1], dt.float32, kind="ExternalOutput").ap()
    import os
    if os.environ.get("KDBG"):
        aps["dbg"] = nc.dram_tensor("dbg", [P, 7], dt.float32,
                                    kind="ExternalOutput").ap()
        for lvl in (0, 1):
            L = LVL[lvl]
            aps[f"dbgk{lvl}"] = nc.dram_tensor(f"dbgk{lvl}", [P, L["SL"]],
                                               dt.uint16, kind="ExternalOutput").ap()
            aps[f"dbgr{lvl}"] = nc.dram_tensor(f"dbgr{lvl}", [P, L["NL"]],
                                               dt.uint16, kind="ExternalOutput").ap()
            aps[f"dbgc{lvl}"] = nc.dram_tensor(f"dbgc{lvl}", [P, NBLK * L["X"]],
                                               dt.float32, kind="ExternalOutput").ap()
            aps[f"dbgw{lvl}"] = nc.dram_tensor(f"dbgw{lvl}", [P, L["NW"]],
                                               dt.float32, kind="ExternalOutput").ap()

    with tile.TileContext(nc) as tc:
        _emit(nc, tc, aps, out_ap)
    nc.compile()
    return nc


def _emit(nc, tc, aps, out_ap):
    import contextlib
    VE, PL, ACT, SP = nc.vector, nc.gpsimd, nc.scalar, nc.sync
    with contextlib.ExitStack() as ctx:
        cpool = ctx.enter_context(tc.tile_pool(name="consts", bufs=1))
        accs = {}
        for name in ("rgb", "inter", "inter1", "p1", "p2", "hash", "hash1"):
            a = cpool.tile([P, 1], dt.float32, tag=f"acc_{name}")
            accs[name] = a

        # ---------- hash loss (emitted first: fills the DMA warmup gap) ----
        ones_h = cpool.tile([P, HCOLS], dt.float32, tag="ones_h")
        PL.memset(ones_h[:], 1.0)
        for lvl in (0, 1):
            with tc.tile_pool(name=f"hash{lvl}", bufs=1) as pool:
                idx = pool.tile([P, HCOLS], dt.uint16, tag="hidx")
                src = aps[f"hi{lvl}"]
                SP.dma_start(idx[:], bass.AP(tensor=src.tensor,
                                             offset=src.offset,
                                             ap=[[HROW, P], [1, HCOLS]]))
                emb = pool.tile([P, HCOLS * 2], dt.float32, tag="hemb")
                esrc = aps[f"he{lvl}"]
                SP.dma_start(emb[:], bass.AP(tensor=esrc.tensor,
                                             offset=esrc.offset,
                                             ap=[[HROW * 2, P], [1, HCOLS * 2]]))
                sq = pool.tile([P, HCOLS * 2], dt.float32, tag="hsq")
                ACT.activation(sq[:], emb[:], ACTF.Square)
                wv = pool.tile([P, HCOLS], dt.float32, tag="hw")
                sq3 = sq[:].rearrange("p (n two) -> p n two", two=2)
                VE.tensor_tensor(wv[:], sq3[:, :, 0], sq3[:, :, 1], Alu.add)
                eq = pool.tile([P, HCOLS], dt.float32, tag="heq")
                PL.memset(eq[:, 0:1], 0.0)
                VE.tensor_tensor(eq[:, 1:HCOLS], idx[:, 1:HCOLS],
                                 idx[:, 0:HCOLS - 1], Alu.is_equal)
                S = pool.tile([P, HCOLS], dt.float32, tag="hS")
                PL.tensor_tensor_scan(S[:], eq[:], wv[:], 0.0, Alu.mult, Alu.add)
                cc = pool.tile([P, HCOLS], dt.float32, tag="hcc")
                PL.tensor_tensor_scan(cc[:], eq[:], ones_h[:], 0.0,
                                      Alu.mult, Alu.add)
                cci = pool.tile([P, HCOLS], dt.float32, tag="hcci")
                VE.reciprocal(cci[:], cc[:])
                ratio = pool.tile([P, HCOLS], dt.float32, tag="hr")
                VE.tensor_tensor(ratio[:], S[:], cci[:], Alu.mult)
                me = pool.tile([P, HCOLS], dt.float32, tag="hme")
                VE.tensor_scalar(me[:, 0:HCOLS - 1], eq[:, 1:HCOLS], -1.0, 1.0,
                                 Alu.mult, Alu.add)
                ttro = pool.tile([P, HROW], dt.float32, tag="httro")
                VE.tensor_tensor_reduce(ttro[:], ratio[:, HALO:HALO + HROW],
                                        me[:, HALO:HALO + HROW], 1.0, 0.0,
                                        Alu.mult, Alu.add,
                                        accs["hash" if lvl == 0 else "hash1"][:])


        # ---------- shared render tables + radio + dist ----------
        spool = ctx.enter_context(tc.tile_pool(name="shared", bufs=1))
        sdrw = spool.tile([P, NBLK * 97], dt.float32, tag="sdrw")
        SP.dma_start(_blk(sdrw[:], 97),
                     aps["sdrw"].rearrange("(b p) x -> p b x", p=P))
        s_sh = _blk(sdrw[:], 97)[:, :, 0:49]
        radios = {0: spool.tile([P, NBLK * 49], dt.float32, tag="radio0",
                                name="radio0"),
                  1: spool.tile([P, NBLK * 49], dt.float32, tag="radio1",
                                name="radio1")}
        b1t = spool.tile([P, 2 * NBLK * 128], dt.uint16, tag="b1t")

        with tc.tile_pool(name="setup", bufs=1) as pool:
            rwv = _blk(sdrw[:], 97)[:, :, 49:97]
            s3 = s_sh
            rw_sh = pool.tile([P, NBLK * 48], dt.float32, tag="rw_sh")
            VE.tensor_copy(_blk(rw_sh[:], 48), rwv)
            ds = pool.tile([P, NBLK * 48], dt.float32, tag="ds")
            VE.tensor_tensor(_blk(ds[:], 48), s3[:, :, 1:49], s3[:, :, 0:48],
                             Alu.subtract)
            dsi = pool.tile([P, NBLK * 48], dt.float32, tag="dsi")
            ACT.activation(dsi[:], ds[:], ACTF.Copy, bias=1e-8)
            VE.reciprocal(dsi[:], dsi[:])
            wnorm = pool.tile([P, NBLK * 48], dt.float32, tag="wnorm")
            VE.tensor_tensor(wnorm[:], rw_sh[:], dsi[:], Alu.mult)
            wnp = pool.tile([P, NBLK * 50], dt.float32, tag="wnp")
            PL.memset(wnp[:], 0.0)
            VE.tensor_copy(_blk(wnp[:], 50)[:, :, 1:49], _blk(wnorm[:], 48))
            diff = pool.tile([P, NBLK * 49], dt.float32, tag="diff")
            wnp3 = _blk(wnp[:], 50)
            VE.tensor_tensor(_blk(diff[:], 49), wnp3[:, :, 1:50],
                             wnp3[:, :, 0:49], Alu.subtract)
            for lvl in (0, 1):
                # 1/(4*pw): includes the 0.5 of the trapezoid area
                VE.tensor_scalar(radios[lvl][:], diff[:], 1.0 / (4 * PULSE[lvl]),
                                 None, Alu.mult)

            # ---------- shared event merge (both levels, 128-wide asc) -----
            # build into b1b, partial d=64 stage into b1t, then six full
            # stages ping-pong back into b1t.
            b1b = pool.tile([P, 2 * NBLK * 128], dt.uint16, tag="b1b")
            b1g = b1b[:].rearrange("p (g n) -> p g n", n=128)
            PL.memset(b1g[:, :, 49:79], PADK)
            for lvl in (0, 1):
                pw = PULSE[lvl]
                kem = pool.tile([P, NBLK * 49], dt.uint16, tag=f"kem{lvl}")
                ACT.activation(_blk(kem[:], 49), s3, ACTF.Copy, scale=S4,
                               bias=(OFF - pw) * S4)
                _ts_int(VE, kem[:], kem[:], 0xFFFC, Alu.bitwise_and, 1,
                        Alu.bitwise_or)
                kep = pool.tile([P, NBLK * 49], dt.uint16, tag=f"kep{lvl}")
                ACT.activation(_blk(kep[:], 49), s3, ACTF.Copy, scale=S4,
                               bias=(OFF + pw) * S4)
                _ts_int(VE, kep[:], kep[:], 0xFFFC, Alu.bitwise_and, 3,
                        Alu.bitwise_or)
                g0 = lvl * NBLK
                VE.tensor_copy(b1g[:, g0:g0 + NBLK, 0:49], _blk(kem[:], 49))
                VE.tensor_copy(b1g[:, g0:g0 + NBLK, 79:128],
                               _blk(kep[:], 49)[:, :, ::-1])
            # partial first stage (d=64): only pairs (15..63, 79..127) matter
            b1n = b1t[:].rearrange("p (g n) -> p g n", n=128)
            VE.tensor_tensor(b1n[:, :, 15:64], b1g[:, :, 15:64],
                             b1g[:, :, 79:128], Alu.min)
            VE.tensor_tensor(b1n[:, :, 79:128], b1g[:, :, 15:64],
                             b1g[:, :, 79:128], Alu.max)
            VE.tensor_copy(b1n[:, :, 0:15], b1g[:, :, 0:15])
            VE.tensor_copy(b1n[:, :, 64:79], b1g[:, :, 64:79])
            res, _ = _merge_stages(VE, b1t, b1b, 128, [32, 16, 8, 4, 2, 1])
            assert res is b1t

            # ---------- distortion ----------
            mask48 = pool.tile([P, NBLK * 48], dt.float32, tag="mask48")
            PL.memset(mask48[:], 1.0)
            PL.memset(_blk(mask48[:], 48)[:, :, 0:1], 0.0)
            mid = pool.tile([P, NBLK * 48], dt.float32, tag="mid")
            VE.tensor_tensor(_blk(mid[:], 48), s3[:, :, 1:49], s3[:, :, 0:48],
                             Alu.add)   # 2*mid; the 0.5 folds into W_DIST
            wm = pool.tile([P, NBLK * 48], dt.float32, tag="wm")
            VE.tensor_tensor(wm[:], rw_sh[:], mid[:], Alu.mult)
            Cin = pool.tile([P, NBLK * 48], dt.float32, tag="Cin")
            PL.tensor_tensor_scan(Cin[:], mask48[:], rw_sh[:], 0.0,
                                  Alu.mult, Alu.add)
            Sin = pool.tile([P, NBLK * 48], dt.float32, tag="Sin")
            PL.tensor_tensor_scan(Sin[:], mask48[:], wm[:], 0.0,
                                  Alu.mult, Alu.add)
            A = pool.tile([P, NBLK * 47], dt.float32, tag="A47")
            m3 = _blk(mid[:], 48)
            c3 = _blk(Cin[:], 48)
            sw3 = _blk(Sin[:], 48)
            rw3 = _blk(rw_sh[:], 48)
            A3 = _blk(A[:], 47)
            VE.tensor_tensor(A3, m3[:, :, 1:48], c3[:, :, 0:47], Alu.mult)
            VE.tensor_tensor(A3, A3, sw3[:, :, 0:47], Alu.subtract)
            ttro = pool.tile([P, NBLK * 47], dt.float32, tag="dttro")
            VE.tensor_tensor_reduce(_blk(ttro[:], 47), A3, rw3[:, :, 1:48],
                                    1.0, 0.0, Alu.mult, Alu.add, accs["p1"][:],
                                    opt_aps=False)
            t2 = pool.tile([P, NBLK * 48], dt.float32, tag="t2d")
            VE.tensor_tensor(t2[:], rw_sh[:], rw_sh[:], Alu.mult)
            ttro2 = pool.tile([P, NBLK * 48], dt.float32, tag="dttro2")
            VE.tensor_tensor_reduce(ttro2[:], t2[:], ds[:], 1.0, 0.0,
                                    Alu.mult, Alu.add, accs["p2"][:])

        # ---------- inter loss (levels interleaved phase-wise) ----------
        lvl_pools = {l: ctx.enter_context(tc.tile_pool(name=f"lvl{l}", bufs=1))
                     for l in (0, 1)}
        sts = {}
        for lvl in (0, 1):
            sts[lvl] = _emit_level_p1(nc, tc, lvl_pools[lvl], lvl, s_sh,
                                      radios[lvl], b1t, aps, accs)
        for lvl in (0, 1):
            _emit_level_p2(nc, tc, lvl_pools[lvl], lvl, sts[lvl], aps, accs)

        # ---------- rgb ----------
        with tc.tile_pool(name="rgb", bufs=1) as pool:
            pdgt = pool.tile([P, NBLK * 6], dt.float32, tag="pdgt")
            SP.dma_start(_blk(pdgt[:], 6),
                         aps["pdgt"].rearrange("(b p) c -> p b c", p=P))
            pg3 = _blk(pdgt[:], 6)
            d = pool.tile([P, NBLK * 3], dt.float32, tag="rgbd")
            VE.tensor_tensor(_blk(d[:], 3), pg3[:, :, 0:3], pg3[:, :, 3:6],
                             Alu.subtract)
            dsq = pool.tile([P, NBLK * 3], dt.float32, tag="rgbsq")
            ACT.activation(dsq[:], d[:], ACTF.Square, accum_out=accs["rgb"][:])


        # ---------- combine + output ----------
        with tc.tile_pool(name="fin", bufs=1) as pool:
            tot = pool.tile([P, 1], dt.float32, tag="tot")
            VE.tensor_scalar(tot[:], accs["rgb"][:], W_RGB / (R * 3), None,
                             Alu.mult)
            VE.scalar_tensor_tensor(tot[:], accs["inter"][:], W_INTER,
                                    tot[:], Alu.mult, Alu.add)
            VE.scalar_tensor_tensor(tot[:], accs["inter1"][:], W_INTER,
                                    tot[:], Alu.mult, Alu.add)
            VE.scalar_tensor_tensor(tot[:], accs["p1"][:], W_DIST / R,
                                    tot[:], Alu.mult, Alu.add)
            VE.scalar_tensor_tensor(tot[:], accs["p2"][:], W_DIST / (3.0 * R),
                                    tot[:], Alu.mult, Alu.add)
            VE.scalar_tensor_tensor(tot[:], accs["hash"][:],
                                    W_HASH / (NUM_SEGMENTS * 2.0), tot[:],
                                    Alu.mult, Alu.add)
            VE.scalar_tensor_tensor(tot[:], accs["hash1"][:],
                                    W_HASH / (NUM_SEGMENTS * 2.0), tot[:],
                                    Alu.mult, Alu.add)
            res = pool.tile([P, 1], dt.float32, tag="res")
            PL.partition_all_reduce(res[:], tot[:], channels=P,
                                    reduce_op=bass_isa.ReduceOp.add)
            SP.dma_start(out_ap, res[0:1, 0:1])
            import os
            if os.environ.get("KDBG") and "dbg" in aps:
                dbg = pool.tile([P, 7], dt.float32, tag="dbg")
                for i, name in enumerate(("rgb", "inter", "inter1", "p1",
                                          "p2", "hash", "hash1")):
                    VE.tensor_copy(dbg[:, i:i + 1], accs[name][:])
                SP.dma_start(aps["dbg"], dbg[:])


# ---------------- host side ----------------
_module_cache = {}


def _get_module():
    if "nc" not in _module_cache:
        _module_cache["nc"] = build_module()
    return _module_cache["nc"]


def shard_inputs(inputs):
    """Full inputs -> list of 8 per-core in_maps."""
    f32 = np.float32
    pd = np.ascontiguousarray(inputs["pd_rgbs"], f32)
    gt = np.ascontiguousarray(inputs["gt_rgbs"], f32)
    sd = np.ascontiguousarray(inputs["render_sdist"], f32)
    rw = np.ascontiguousarray(inputs["render_weights"], f32)
    ps0 = np.ascontiguousarray(inputs["prop_sdist_0"], f32)
    pw0 = np.ascontiguousarray(inputs["prop_weights_0"], f32)
    ps1 = np.ascontiguousarray(inputs["prop_sdist_1"], f32)
    pw1 = np.ascontiguousarray(inputs["prop_weights_1"], f32)
    hashes = {}
    for lvl in (0, 1):
        idx = np.asarray(inputs[f"enc_idx_{lvl}"]).astype(np.int64)
        emb = np.ascontiguousarray(inputs[f"enc_embds_{lvl}"], f32)
        idx_pad = np.empty(M + 2 * HALO, np.uint16)
        idx_pad[HALO:HALO + M] = idx.astype(np.uint16)
        # pads must differ from the adjacent real idx (run-break sentinels)
        idx_pad[:HALO] = np.uint16((int(idx[0]) + 1) & 0xFFFF)
        idx_pad[HALO + M:] = np.uint16((int(idx[-1]) + 1) & 0xFFFF)
        emb_pad = np.zeros((M + 2 * HALO, 2), f32)
        emb_pad[HALO:HALO + M] = emb
        hashes[lvl] = (idx_pad, emb_pad)

    consts = {}
    for lvl, L in LVL.items():
        LW, QWS, X = L["LW"], L["QWS"], L["X"]
        p1 = np.tile(np.arange(1, LW + 1, dtype=np.int16), NBLK)
        ic = np.concatenate([np.arange(1, LW + 1, dtype=np.int16) + b * QWS
                             for b in range(NBLK)])
        xl = np.concatenate([np.arange(X, dtype=np.int16) + b * LW
                             for b in range(NBLK)])
        row = np.concatenate([ic, xl])
        consts[f"c_iota_l{lvl}"] = np.ascontiguousarray(np.tile(row, (P, 1)))

    pdgt = np.concatenate([pd, gt], axis=1)
    sdrw = np.concatenate([sd, rw], axis=1)
    pspw = {0: np.concatenate([ps0, pw0], axis=1),
            1: np.concatenate([ps1, pw1], axis=1)}
    pslh = {}
    for lvl, ps in ((0, ps0), (1, ps1)):
        pu = ps.view(np.uint16).reshape(R, -1, 2)
        pslh[lvl] = (np.ascontiguousarray(pu[:, :, 0]),
                     np.ascontiguousarray(pu[:, :, 1]))

    in_maps = []
    for c in range(N_CORES):
        r0 = c * RPC
        lo = c * MPC
        im = {
            "pdgt": pdgt[r0:r0 + RPC],
            "sdrw": sdrw[r0:r0 + RPC],
            "pspw0": pspw[0][r0:r0 + RPC], "pspw1": pspw[1][r0:r0 + RPC],
            "pslo0": pslh[0][0][r0:r0 + RPC], "pshi0": pslh[0][1][r0:r0 + RPC],
            "pslo1": pslh[1][0][r0:r0 + RPC], "pshi1": pslh[1][1][r0:r0 + RPC],
        }
        for lvl in (0, 1):
            idx_pad, emb_pad = hashes[lvl]
            im[f"hi{lvl}"] = np.ascontiguousarray(idx_pad[lo:lo + HSLICE])
            im[f"he{lvl}"] = np.ascontiguousarray(
                emb_pad[lo:lo + HSLICE].reshape(-1))
        im.update(consts)
        in_maps.append(im)
    return in_maps


def kernel(**inputs) -> np.ndarray:
    nc = _get_module()
    in_maps = shard_inputs(inputs)
    res = run_bass_kernel_spmd(nc, in_maps, core_ids=list(range(N_CORES)))
    total = np.float64(0.0)
    for r in res.results:
        total += np.float64(r["out"][0, 0])
    return np.float32(total)


# revision 41
# speedup vs baseline: 1.1218x; 1.1218x over previous
"""Trainium2 Bass kernel for nn_Loss_dict_50646254354805 (NeRF-style loss).

Self-contained: accepts FULL inputs, shards across 8 NeuronCores (rays for
the per-ray losses, samples for the hash loss), runs one SPMD Bass module,
host-sums the 8 partial scalars.

Inter-loss: the reference's blur_step_function + sorted_interp_quad is
evaluated in a merged domain. Keys are uint16 quantized values (14-bit grid)
with 2-bit source tags, bitonic-merged at 2x DVE rate; per-slot values come
from the keys (grid error ~6e-5, validated ~1e-2 rel on the inter terms,
~0.5% on the total loss vs 2e-2 budget); the +-radio slopes are scattered
as exact f32 halves (their telescoping cancellation needs full precision).
Density/CDF reconstruction runs as masked prefix scans on the Pool engine;
conversions/relu/square run on the Activation engine; counts, positions and
compaction indices are uint16 DVE ops at 2-4x rate.
"""
import numpy as np

import concourse.bass as bass
import concourse.bass_isa as bass_isa
import concourse.mybir as mybir
import concourse.tile as tile
from concourse import bacc
from concourse.bass_utils import run_bass_kernel_spmd

dt = mybir.dt
Alu = mybir.AluOpType
AX = mybir.AxisListType
ACTF = mybir.ActivationFunctionType
P = 128

# problem constants
PULSE = (0.01, 0.005)
W_RGB, W_INTER, W_DIST, W_HASH = 1.0, 1.0, 0.01, 0.1
NUM_SEGMENTS = 65536
R, N = 4096, 48
M = R * N
N_CORES = 8
RPC = R // N_CORES            # rays per core (512)
NBLK = RPC // P               # ray tiles per core (4)
MPC = M // N_CORES            # hash samples per core (24576)
HALO = 64                     # hash run halo
HROW = MPC // P               # hash samples per partition (192)
HCOLS = HROW + HALO + 1       # loaded cols per partition (257)
HSLICE = HALO + MPC + HALO    # per-core hash slice length (24704)

# key quantization: key = trunc((v + OFF) * S4), tags in the low 2 bits
S4 = 63000.0
OFF = 0.02
PADK = 0xFFFC                 # pad key (tag 0, larger than any real key)

# per-level geometry
LVL = {0: dict(X=257, n2=512), 1: dict(X=97, n2=256)}
for _L in LVL.values():
    _L["EW"] = ((_L["X"] + 98 + 1 + 7) // 8) * 8        # 360 / 200
    _L["LW"] = _L["EW"] + 24                            # 384 / 224
    _L["NL"] = NBLK * _L["LW"]                          # 1536 / 896
    _L["SL"] = NBLK * _L["n2"]                          # 2048 / 1024
    _L["NW"] = NBLK * (_L["X"] - 1)                     # 1024 / 384
    _L["QWS"] = _L["LW"] - 98                           # 286 / 126
    _L["NQ"] = NBLK * _L["QWS"]


def _ts_int(eng, out, in0, imm1, op0, imm2=None, op1=None):
    """tensor_scalar with int32 immediates (for bitwise/compare ops)."""
    ins_ = [eng.lower_ap(in0), mybir.ImmediateValue(dtype=dt.int32, value=int(imm1))]
    kw = dict(op0=op0)
    if imm2 is not None:
        ins_.append(mybir.ImmediateValue(dtype=dt.int32, value=int(imm2)))
        kw["op1"] = op1
    return eng.add_instruction(mybir.InstTensorScalarPtr(
        name=eng.bass.get_next_instruction_name(),
        ins=ins_, outs=[eng.lower_ap(out)], **kw))


def _blk(ap, n2):
    return ap.rearrange("p (b n) -> p b n", b=NBLK)


def _merge_stages(VE, bufa, bufb, width, d_list, descending=False):
    """Full bitonic merge stages (ping-pong) over [P, G*width] u16 tiles."""
    cur, nxt = bufa, bufb
    for d in d_list:
        c3 = cur[:].rearrange("p (c td) -> p c td", td=2 * d)
        n3 = nxt[:].rearrange("p (c td) -> p c td", td=2 * d)
        lo_in, hi_in = c3[:, :, 0:d], c3[:, :, d:2 * d]
        if descending:
            VE.tensor_tensor(n3[:, :, 0:d], lo_in, hi_in, Alu.max)
            VE.tensor_tensor(n3[:, :, d:2 * d], lo_in, hi_in, Alu.min)
        else:
            VE.tensor_tensor(n3[:, :, 0:d], lo_in, hi_in, Alu.min)
            VE.tensor_tensor(n3[:, :, d:2 * d], lo_in, hi_in, Alu.max)
        cur, nxt = nxt, cur
    return cur, nxt


def _emit_level_p1(nc, tc, pool, lvl, s_sh, radio, b1t, aps, accs):
    """Phase 1: merge, flags/counts, scatters, exact values, radio."""
    VE, PL, ACT, SP = nc.vector, nc.gpsimd, nc.scalar, nc.sync
    L = LVL[lvl]
    X, n2, EW, LW, NL, SL, NW, QWS, NQ = (L["X"], L["n2"], L["EW"], L["LW"],
                                          L["NL"], L["SL"], L["NW"], L["QWS"],
                                          L["NQ"])
    pw = PULSE[lvl]

    def blkL(ap):
        return ap.rearrange("p (b n) -> p b n", b=NBLK)

    st = dict(blkL=blkL)

    # ---------- per-level constants (one batched DMA on SP) ----------
    iotas = pool.tile([P, NL + NBLK * X], dt.int16, tag="iotas")
    SP.dma_start(iotas[:], aps[f"c_iota_l{lvl}"][:, 0:NL + NBLK * X])
    iotaC = iotas[:][:, 0:NL]
    aps_iotaxl = iotas[:][:, NL:]
    maskf = pool.tile([P, NL], dt.float32, tag="maskf")
    PL.memset(maskf[:], 1.0)
    PL.memset(blkL(maskf[:])[:, :, 0:1], 0.0)
    st["maskf"] = maskf

    # ---------- inputs (one batched DMA: [ps | pw] per ray) ----------
    pspw = pool.tile([P, NBLK * (2 * X - 1)], dt.float32, tag="pspw")
    SP.dma_start(_blk(pspw[:], 2 * X - 1),
                 aps[f"pspw{lvl}"].rearrange("(b p) x -> p b x", p=P))
    xt3 = _blk(pspw[:], 2 * X - 1)[:, :, 0:X]
    pwt3 = _blk(pspw[:], 2 * X - 1)[:, :, X:2 * X - 1]
    st["pwt3"] = pwt3
    dinv = pool.tile([P, NW], dt.float32, tag="dinv")
    ACT.activation(_blk(dinv[:], X - 1), pwt3, ACTF.Copy, bias=1e-5)
    VE.reciprocal(dinv[:], dinv[:])
    st["dinv"] = dinv

    # ---------- big merge: queries + events (from b1t), ascending ----------
    B0 = pool.tile([P, SL], dt.uint16, tag="big0")
    B1 = pool.tile([P, SL], dt.uint16, tag="big1")
    b03 = _blk(B0[:], n2)
    PL.memset(b03[:, :, X:n2 - 128], PADK)
    # quantized query keys written straight into the merge buffer
    ACT.activation(b03[:, :, 0:X], xt3, ACTF.Copy, scale=S4, bias=OFF * S4)
    _ts_int(VE, b03[:, :, 0:X], b03[:, :, 0:X], 0xFFFC, Alu.bitwise_and)
    b1f = b1t[:].rearrange("p (g n) -> p g n", n=128)
    g0 = lvl * NBLK
    VE.tensor_copy(b03[:, :, n2 - 128:n2], b1f[:, g0:g0 + NBLK, ::-1])
    # first stage: only the trailing 98 pairs touch real data
    d0 = n2 // 2
    VE.tensor_tensor(_blk(B1[:], n2)[:, :, d0 - 98:d0],
                     b03[:, :, d0 - 98:d0], b03[:, :, n2 - 98:n2], Alu.min)
    VE.tensor_tensor(_blk(B1[:], n2)[:, :, n2 - 98:n2],
                     b03[:, :, d0 - 98:d0], b03[:, :, n2 - 98:n2], Alu.max)
    VE.tensor_copy(_blk(B1[:], n2)[:, :, 0:d0 - 98], b03[:, :, 0:d0 - 98])
    VE.tensor_copy(_blk(B1[:], n2)[:, :, d0:n2 - 98], b03[:, :, d0:n2 - 98])
    ds_rest = [n2 // 4]
    while ds_rest[-1] > 1:
        ds_rest.append(ds_rest[-1] // 2)
    Kt, Ksc = _merge_stages(VE, B1, B0, n2, ds_rest)
    mS = _blk(Kt[:], n2)[:, :, 0:LW]       # merged keys, strided [P,NBLK,LW]
    st["mS"] = mS
    st["Ksc"] = Ksc

    # ---------- flags / counts (u16) ----------
    ev16 = pool.tile([P, NL], dt.uint16, tag="ev16")
    _ts_int(VE, blkL(ev16[:]), mS, 1, Alu.bitwise_and)
    em16 = pool.tile([P, NL], dt.uint16, tag="em16")
    _ts_int(VE, blkL(em16[:]), mS, 3, Alu.bitwise_and, 1, Alu.is_equal)
    ep16 = pool.tile([P, NL], dt.uint16, tag="ep16")
    _ts_int(VE, blkL(ep16[:]), mS, 3, Alu.bitwise_and, 3, Alu.is_equal)
    C16 = pool.tile([P, NL], dt.uint16, tag="C16")
    PL.tensor_tensor_scan(C16[:], maskf[:], ev16[:], 0.0, Alu.mult, Alu.add)
    Cm16 = pool.tile([P, NL], dt.uint16, tag="Cm16")
    PL.tensor_tensor_scan(Cm16[:], maskf[:], em16[:], 0.0, Alu.mult, Alu.add)
    st["ev16"] = ev16
    st["ep16"] = ep16

    # ---------- event position scatters ----------
    tmp16 = pool.tile([P, NL], dt.uint16, tag="tmp16")
    idx16 = pool.tile([P, NL], dt.int16, tag="idx16")
    t3 = blkL(tmp16[:])
    i3 = blkL(idx16[:])
    C3, Cm3, em3, ep3 = (blkL(C16[:]), blkL(Cm16[:]), blkL(em16[:]),
                         blkL(ep16[:]))
    pos_m = pool.tile([P, NBLK * 64], dt.uint16, tag="pos_m")
    pos_p = pool.tile([P, NBLK * 64], dt.uint16, tag="pos_p")
    for which, pos in ((0, pos_m), (1, pos_p)):
        if which == 0:
            VE.tensor_tensor(t3[:, :, 0:EW], Cm3[:, :, 0:EW], em3[:, :, 0:EW],
                             Alu.mult)
        else:
            VE.tensor_tensor(t3[:, :, 0:EW], C3[:, :, 0:EW], Cm3[:, :, 0:EW],
                             Alu.subtract)
            VE.tensor_tensor(t3[:, :, 0:EW], t3[:, :, 0:EW], ep3[:, :, 0:EW],
                             Alu.mult)
        _ts_int(VE, i3[:, :, 0:EW], t3[:, :, 0:EW], -1, Alu.add)
        for b in range(NBLK):
            PL.local_scatter(pos[:, b * 64:(b + 1) * 64],
                             iotaC[:, b * LW:b * LW + EW].bitcast(dt.uint16),
                             idx16[:, b * LW:b * LW + EW], channels=P,
                             num_elems=64, num_idxs=EW)

    # ---------- radio scatter (exact f32 halves) ----------
    CW = NBLK * 128 + NBLK * X
    idxcat = pool.tile([P, CW], dt.int16, tag="idxcat")
    tgt16 = idxcat[:][:, 0:NBLK * 128]
    tg3 = _blk(tgt16, 128)
    pm3 = _blk(pos_m[:], 64)
    pp3 = _blk(pos_p[:], 64)
    for b in range(NBLK):
        # iotaC data carries +b*QWS; fold its removal into the block offset
        _ts_int(VE, tg3[:, b, 0:49], pm3[:, b, 0:49], b * (LW - QWS) - 1, Alu.add)
        _ts_int(VE, tg3[:, b, 49:98], pp3[:, b, 0:49], b * (LW - QWS) - 1, Alu.add)
    PL.memset(tg3[:, :, 98:128], -1)

    radcat = pool.tile([P, NBLK * 128], dt.float32, tag="radcat")
    r3 = _blk(radcat[:], 128)
    VE.tensor_copy(r3[:, :, 0:49], _blk(radio[:], 49))
    VE.tensor_scalar(r3[:, :, 49:98], _blk(radio[:], 49), -1.0, None, Alu.mult)
    PL.memset(r3[:, :, 98:128], 0.0)
    rc_u = radcat[:].bitcast(dt.uint16).rearrange("p (n two) -> p n two", two=2)
    rad_lo = pool.tile([P, NBLK * 128], dt.uint16, tag="rad_lo")
    rad_hi = pool.tile([P, NBLK * 128], dt.uint16, tag="rad_hi")
    VE.tensor_copy(rad_lo[:], rc_u[:, :, 0])
    VE.tensor_copy(rad_hi[:], rc_u[:, :, 1])
    rl_t = pool.tile([P, NL], dt.uint16, tag="rl_t")
    rh_t = pool.tile([P, NL], dt.uint16, tag="rh_t")
    PL.local_scatter(rl_t[:], rad_lo[:], tgt16, channels=P,
                     num_elems=NL, num_idxs=NBLK * 128)
    PL.local_scatter(rh_t[:], rad_hi[:], tgt16, channels=P,
                     num_elems=NL, num_idxs=NBLK * 128)
    radio_m = pool.tile([P, NL], dt.float32, tag="radio_m")
    rm_u = radio_m[:].bitcast(dt.uint16).rearrange("p (n two) -> p n two", two=2)
    ACT.activation(rm_u[:, :, 0], rl_t[:], ACTF.Copy)
    ACT.activation(rm_u[:, :, 1], rh_t[:], ACTF.Copy)
    st["radio_m"] = radio_m

    # ---------- compaction indices (reused later for the cdf compact) ------
    qf16 = em16                           # em16 dead after pos idx
    _ts_int(VE, blkL(qf16[:]), mS, 3, Alu.bitwise_and, 0, Alu.is_equal)
    tq = tmp16                            # tmp16 dead after pos idx
    VE.tensor_tensor(tq[:], iotaC.bitcast(dt.uint16), C16[:], Alu.subtract)
    VE.tensor_tensor(tq[:], tq[:], qf16[:], Alu.mult)
    idxq = pool.tile([P, NL], dt.int16, tag="idxq")
    _ts_int(VE, idxq[:], tq[:], -1, Alu.add)
    st["idxq"] = idxq

    # ---------- exact per-slot values (queries + events, one scatter) ------
    i0q = qf16                            # qf16 dead after idxq
    PL.local_scatter(i0q[:, 0:NQ], C16[:], idxq[:], channels=P,
                     num_elems=NQ, num_idxs=NL)
    VE.tensor_tensor(_blk(idxcat[:][:, NBLK * 128:CW], X).bitcast(dt.uint16),
                     aps_iotaxl.bitcast(dt.uint16).rearrange(
                         "p (b n) -> p b n", b=NBLK),
                     _blk(i0q[:, 0:NQ], QWS)[:, :, 0:X], Alu.add)
    emsh = pool.tile([P, NBLK * 49], dt.float32, tag="emsh")
    ACT.activation(_blk(emsh[:], 49), s_sh, ACTF.Copy, bias=-pw)
    epsh = pool.tile([P, NBLK * 49], dt.float32, tag="epsh")
    ACT.activation(_blk(epsh[:], 49), s_sh, ACTF.Copy, bias=pw)
    vc_lo = pool.tile([P, CW], dt.uint16, tag="vc_lo")
    vc_hi = pool.tile([P, CW], dt.uint16, tag="vc_hi")
    em_u = emsh[:].bitcast(dt.uint16).rearrange("p (b n two) -> p b n two",
                                                b=NBLK, two=2)
    ep_u = epsh[:].bitcast(dt.uint16).rearrange("p (b n two) -> p b n two",
                                                b=NBLK, two=2)
    for half, vc, hname in ((0, vc_lo, "pslo"), (1, vc_hi, "pshi")):
        vch = _blk(vc[:][:, 0:NBLK * 128], 128)
        VE.tensor_copy(vch[:, :, 0:49], em_u[:, :, :, half])
        VE.tensor_copy(vch[:, :, 49:98], ep_u[:, :, :, half])
        PL.memset(vch[:, :, 98:128], 0)
        SP.dma_start(_blk(vc[:][:, NBLK * 128:CW], X),
                     aps[f"{hname}{lvl}"].rearrange("(b p) x -> p b x", p=P))
    vl_t = pool.tile([P, NL], dt.uint16, tag="vl_t")
    vh_t = pool.tile([P, NL], dt.uint16, tag="vh_t")
    PL.local_scatter(vl_t[:], vc_lo[:], idxcat[:], channels=P,
                     num_elems=NL, num_idxs=CW)
    PL.local_scatter(vh_t[:], vc_hi[:], idxcat[:], channels=P,
                     num_elems=NL, num_idxs=CW)
    v = pool.tile([P, NL], dt.float32, tag="v")
    v_u = v[:].bitcast(dt.uint16).rearrange("p (n two) -> p n two", two=2)
    ACT.activation(v_u[:, :, 0], vl_t[:], ACTF.Copy)
    ACT.activation(v_u[:, :, 1], vh_t[:], ACTF.Copy)
    dv = pool.tile([P, NL], dt.float32, tag="dv")
    dv3 = blkL(dv[:])
    v3 = blkL(v[:])
    VE.tensor_tensor(dv3[:, :, 1:EW], v3[:, :, 1:EW], v3[:, :, 0:EW - 1],
                     Alu.subtract)
    st["dv"] = dv
    st["v"] = v
    st["vl_t"] = vl_t
    st["vh_t"] = vh_t
    return st


def _emit_level_p2(nc, tc, pool, lvl, st, aps, accs):
    """Phase 2: density chain, cdf compaction, loss tail."""
    VE, PL, ACT, SP = nc.vector, nc.gpsimd, nc.scalar, nc.sync
    L = LVL[lvl]
    X, EW, LW, NL, NW, QWS, NQ = (L["X"], L["EW"], L["LW"], L["NL"], L["NW"],
                                  L["QWS"], L["NQ"])
    blkL = st["blkL"]
    maskf, radio_m, dv, dinv = st["maskf"], st["radio_m"], st["dv"], st["dinv"]
    idxq, Ksc = st["idxq"], st["Ksc"]
    dv3 = blkL(dv[:])

    # ---------- density chain (g on Pool; w/cdf scans on DVE: the tail
    # window has DVE headroom and DVE scans are cheaper) ----------
    g = pool.tile([P, NL], dt.float32, tag="g")
    PL.tensor_tensor_scan(g[:], maskf[:], radio_m[:], 0.0, Alu.mult, Alu.add)
    wg = radio_m                          # radio_m dead after g scan
    wg3 = blkL(wg[:])
    PL.memset(wg3[:, :, 0:1], 0.0)
    PL.memset(wg3[:, :, EW:LW], 0.0)
    VE.tensor_tensor(wg3[:, :, 1:EW], dv3[:, :, 1:EW], blkL(g[:])[:, :, 0:EW - 1],
                     Alu.mult)
    w = pool.tile([P, NL], dt.float32, tag="w")
    VE.tensor_tensor_scan(w[:], maskf[:], wg[:], 0.0, Alu.mult, Alu.add)
    wc = w                                # relu in place (DVE, no hop)
    VE.tensor_scalar(wc[:], w[:], 0.0, None, Alu.max)
    scr = g                               # g dead after wg
    scr3 = blkL(scr[:])
    wc3 = blkL(wc[:])
    VE.tensor_tensor(scr3[:, :, 1:EW], wc3[:, :, 1:EW], wc3[:, :, 0:EW - 1],
                     Alu.add)
    area = wg                             # wg dead after w scan
    a3 = blkL(area[:])
    PL.memset(a3[:, :, 0:1], 0.0)
    PL.memset(a3[:, :, EW:LW], 0.0)
    # 0.5 of the trapezoid is pre-folded into the radio scale (1/(4*pw))
    VE.tensor_tensor(a3[:, :, 1:EW], scr3[:, :, 1:EW], dv3[:, :, 1:EW], Alu.mult)
    cdf = dv                              # dv dead after area
    VE.tensor_tensor_scan(cdf[:], maskf[:], area[:], 0.0, Alu.mult, Alu.add)

    # ---------- compact cdf at query slots ----------
    cdf_lo = st["ev16"]                   # dead after C scan
    cdf_hi = st["ep16"]                   # dead after pos idx
    cdf_u = cdf[:].bitcast(dt.uint16).rearrange("p (n two) -> p n two", two=2)
    VE.tensor_copy(cdf_lo[:], cdf_u[:, :, 0])
    ACT.activation(cdf_hi[:], cdf_u[:, :, 1], ACTF.Copy)
    cq_lo = st["vl_t"]                    # dead after v recombine
    cq_hi = st["vh_t"]
    PL.local_scatter(cq_lo[:, 0:NQ], cdf_lo[:], idxq[:], channels=P,
                     num_elems=NQ, num_idxs=NL)
    PL.local_scatter(cq_hi[:, 0:NQ], cdf_hi[:], idxq[:], channels=P,
                     num_elems=NQ, num_idxs=NL)
    cdfq = pool.tile([P, NBLK * X], dt.float32, tag="cdfq")
    cq_u = cdfq[:].bitcast(dt.uint16).rearrange("p (b n two) -> p b n two",
                                                b=NBLK, two=2)
    VE.tensor_copy(cq_u[:, :, 0:X, 0], _blk(cq_lo[:, 0:NQ], QWS)[:, :, 0:X])
    ACT.activation(cq_u[:, :, 0:X, 1], _blk(cq_hi[:, 0:NQ], QWS)[:, :, 0:X],
                   ACTF.Copy)

    # ---------- loss tail (all DVE: shortest cross-engine chain) ----------
    ws = cdf[:][:, 0:NW]                  # cdf dead after split
    cqf = _blk(cdfq[:], X)
    ws3 = ws.rearrange("p (b n) -> p b n", b=NBLK)
    VE.tensor_tensor(ws3, cqf[:, :, 1:X], cqf[:, :, 0:X - 1], Alu.subtract)
    t = wc[:][:, 0:NW]                    # wc dead after scr
    VE.tensor_tensor(t.rearrange("p (b n) -> p b n", b=NBLK), ws3, st["pwt3"],
                     Alu.subtract)
    r = scr[:][:, 0:NW]                   # scr dead after area
    ACT.activation(r, t, ACTF.Relu)       # concurrent with u on DVE
    u = area[:][:, 0:NW]                  # area dead after cdf scan
    VE.tensor_tensor(u, t, dinv[:], Alu.mult)
    ttro = Ksc[:].bitcast(dt.float32)[:, 0:NW]   # merge scratch, long dead
    VE.tensor_tensor_reduce(ttro, u, r, 1.0 / (R * (X - 1)), 0.0,
                            Alu.mult, Alu.add,
                            accs["inter" if lvl == 0 else "inter1"][:])


def build_module():
    nc = bacc.Bacc("TRN2", target_bir_lowering=False, debug=False,
                   enable_asserts=False, num_devices=N_CORES)
    aps = {}

    def din(name, shape, dtype=dt.float32):
        aps[name] = nc.dram_tensor(name, shape, dtype, kind="ExternalInput").ap()
    din("pdgt", [RPC, 6])
    din("sdrw", [RPC, 97])
    din("pspw0", [RPC, 513]); din("pspw1", [RPC, 193])
    din("pslo0", [RPC, 257], dt.uint16); din("pshi0", [RPC, 257], dt.uint16)
    din("pslo1", [RPC, 97], dt.uint16); din("pshi1", [RPC, 97], dt.uint16)
    din("hi0", [HSLICE], dt.uint16); din("he0", [HSLICE * 2])
    din("hi1", [HSLICE], dt.uint16); din("he1", [HSLICE * 2])
    for lvl in (0, 1):
        nl = LVL[lvl]["NL"]
        din(f"c_iota_l{lvl}", [P, nl + NBLK * LVL[lvl]["X"]], dt.int16)
    out_ap = nc.dram_tensor("out", [1, 1], dt.float32, kind="ExternalOutput").ap()
    import os
    if os.environ.get("KDBG"):
        aps["dbg"] = nc.dram_tensor("dbg", [P, 7], dt.float32,
                                    kind="ExternalOutput").ap()
        for lvl in (0, 1):
            L = LVL[lvl]
            aps[f"dbgk{lvl}"] = nc.dram_tensor(f"dbgk{lvl}", [P, L["SL"]],
                                               dt.uint16, kind="ExternalOutput").ap()
            aps[f"dbgr{lvl}"] = nc.dram_tensor(f"dbgr{lvl}", [P, L["NL"]],
                                               dt.uint16, kind="ExternalOutput").ap()
            aps[f"dbgc{lvl}"] = nc.dram_tensor(f"dbgc{lvl}", [P, NBLK * L["X"]],
                                               dt.float32, kind="ExternalOutput").ap()
            aps[f"dbgw{lvl}"] = nc.dram_tensor(f"dbgw{lvl}", [P, L["NW"]],
                                               dt.float32, kind="ExternalOutput").ap()

    with tile.TileContext(nc) as tc:
        _emit(nc, tc, aps, out_ap)
    nc.compile()
    return nc


def _emit(nc, tc, aps, out_ap):
    import contextlib
    VE, PL, ACT, SP = nc.vector, nc.gpsimd, nc.scalar, nc.sync
    with contextlib.ExitStack() as ctx:
        cpool = ctx.enter_context(tc.tile_pool(name="consts", bufs=1))
        accs = {}
        for name in ("rgb", "inter", "inter1", "p1", "p2", "hash", "hash1"):
            a = cpool.tile([P, 1], dt.float32, tag=f"acc_{name}")
            accs[name] = a

        # ---------- shared render tables + radio + dist ----------
        spool = ctx.enter_context(tc.tile_pool(name="shared", bufs=1))
        sdrw = spool.tile([P, NBLK * 97], dt.float32, tag="sdrw")
        SP.dma_start(_blk(sdrw[:], 97),
                     aps["sdrw"].rearrange("(b p) x -> p b x", p=P))
        s_sh = _blk(sdrw[:], 97)[:, :, 0:49]
        radios = {0: spool.tile([P, NBLK * 49], dt.float32, tag="radio0",
                                name="radio0"),
                  1: spool.tile([P, NBLK * 49], dt.float32, tag="radio1",
                                name="radio1")}
        b1t = spool.tile([P, 2 * NBLK * 128], dt.uint16, tag="b1t")

        with tc.tile_pool(name="setup", bufs=1) as pool:
            rwv = _blk(sdrw[:], 97)[:, :, 49:97]
            s3 = s_sh
            rw_sh = pool.tile([P, NBLK * 48], dt.float32, tag="rw_sh")
            VE.tensor_copy(_blk(rw_sh[:], 48), rwv)
            ds = pool.tile([P, NBLK * 48], dt.float32, tag="ds")
            VE.tensor_tensor(_blk(ds[:], 48), s3[:, :, 1:49], s3[:, :, 0:48],
                             Alu.subtract)
            dsi = pool.tile([P, NBLK * 48], dt.float32, tag="dsi")
            ACT.activation(dsi[:], ds[:], ACTF.Copy, bias=1e-8)
            VE.reciprocal(dsi[:], dsi[:])
            wnorm = pool.tile([P, NBLK * 48], dt.float32, tag="wnorm")
            VE.tensor_tensor(wnorm[:], rw_sh[:], dsi[:], Alu.mult)
            wnp = pool.tile([P, NBLK * 50], dt.float32, tag="wnp")
            PL.memset(wnp[:], 0.0)
            VE.tensor_copy(_blk(wnp[:], 50)[:, :, 1:49], _blk(wnorm[:], 48))
            diff = pool.tile([P, NBLK * 49], dt.float32, tag="diff")
            wnp3 = _blk(wnp[:], 50)
            VE.tensor_tensor(_blk(diff[:], 49), wnp3[:, :, 1:50],
                             wnp3[:, :, 0:49], Alu.subtract)
            for lvl in (0, 1):
                # 1/(4*pw): includes the 0.5 of the trapezoid area
                VE.tensor_scalar(radios[lvl][:], diff[:], 1.0 / (4 * PULSE[lvl]),
                                 None, Alu.mult)

            # ---------- shared event merge (both levels, 128-wide asc) -----
            # build into b1b, partial d=64 stage into b1t, then six full
            # stages ping-pong back into b1t.
            b1b = pool.tile([P, 2 * NBLK * 128], dt.uint16, tag="b1b")
            b1g = b1b[:].rearrange("p (g n) -> p g n", n=128)
            PL.memset(b1g[:, :, 49:79], PADK)
            for lvl in (0, 1):
                pw = PULSE[lvl]
                kem = pool.tile([P, NBLK * 49], dt.uint16, tag=f"kem{lvl}")
                ACT.activation(_blk(kem[:], 49), s3, ACTF.Copy, scale=S4,
                               bias=(OFF - pw) * S4)
                _ts_int(VE, kem[:], kem[:], 0xFFFC, Alu.bitwise_and, 1,
                        Alu.bitwise_or)
                kep = pool.tile([P, NBLK * 49], dt.uint16, tag=f"kep{lvl}")
                ACT.activation(_blk(kep[:], 49), s3, ACTF.Copy, scale=S4,
                               bias=(OFF + pw) * S4)
                _ts_int(VE, kep[:], kep[:], 0xFFFC, Alu.bitwise_and, 3,
                        Alu.bitwise_or)
                g0 = lvl * NBLK
                VE.tensor_copy(b1g[:, g0:g0 + NBLK, 0:49], _blk(kem[:], 49))
                VE.tensor_copy(b1g[:, g0:g0 + NBLK, 79:128],
                               _blk(kep[:], 49)[:, :, ::-1])
            # partial first stage (d=64): only pairs (15..63, 79..127) matter
            b1n = b1t[:].rearrange("p (g n) -> p g n", n=128)
            VE.tensor_tensor(b1n[:, :, 15:64], b1g[:, :, 15:64],
                             b1g[:, :, 79:128], Alu.min)
            VE.tensor_tensor(b1n[:, :, 79:128], b1g[:, :, 15:64],
                             b1g[:, :, 79:128], Alu.max)
            VE.tensor_copy(b1n[:, :, 0:15], b1g[:, :, 0:15])
            VE.tensor_copy(b1n[:, :, 64:79], b1g[:, :, 64:79])
            res, _ = _merge_stages(VE, b1t, b1b, 128, [32, 16, 8, 4, 2, 1])
            assert res is b1t

            # ---------- distortion ----------
            mask48 = pool.tile([P, NBLK * 48], dt.float32, tag="mask48")
            PL.memset(mask48[:], 1.0)
            PL.memset(_blk(mask48[:], 48)[:, :, 0:1], 0.0)
            mid = pool.tile([P, NBLK * 48], dt.float32, tag="mid")
            VE.tensor_tensor(_blk(mid[:], 48), s3[:, :, 1:49], s3[:, :, 0:48],
                             Alu.add)   # 2*mid; the 0.5 folds into W_DIST
            wm = pool.tile([P, NBLK * 48], dt.float32, tag="wm")
            VE.tensor_tensor(wm[:], rw_sh[:], mid[:], Alu.mult)
            Cin = pool.tile([P, NBLK * 48], dt.float32, tag="Cin")
            PL.tensor_tensor_scan(Cin[:], mask48[:], rw_sh[:], 0.0,
                                  Alu.mult, Alu.add)
            Sin = pool.tile([P, NBLK * 48], dt.float32, tag="Sin")
            PL.tensor_tensor_scan(Sin[:], mask48[:], wm[:], 0.0,
                                  Alu.mult, Alu.add)
            A = pool.tile([P, NBLK * 47], dt.float32, tag="A47")
            m3 = _blk(mid[:], 48)
            c3 = _blk(Cin[:], 48)
            sw3 = _blk(Sin[:], 48)
            rw3 = _blk(rw_sh[:], 48)
            A3 = _blk(A[:], 47)
            VE.tensor_tensor(A3, m3[:, :, 1:48], c3[:, :, 0:47], Alu.mult)
            VE.tensor_tensor(A3, A3, sw3[:, :, 0:47], Alu.subtract)
            ttro = pool.tile([P, NBLK * 47], dt.float32, tag="dttro")
            VE.tensor_tensor_reduce(_blk(ttro[:], 47), A3, rw3[:, :, 1:48],
                                    1.0, 0.0, Alu.mult, Alu.add, accs["p1"][:],
                                    opt_aps=False)
            t2 = pool.tile([P, NBLK * 48], dt.float32, tag="t2d")
            VE.tensor_tensor(t2[:], rw_sh[:], rw_sh[:], Alu.mult)
            ttro2 = pool.tile([P, NBLK * 48], dt.float32, tag="dttro2")
            VE.tensor_tensor_reduce(ttro2[:], t2[:], ds[:], 1.0, 0.0,
                                    Alu.mult, Alu.add, accs["p2"][:])

        # ---------- inter loss (levels interleaved phase-wise) ----------
        lvl_pools = {l: ctx.enter_context(tc.tile_pool(name=f"lvl{l}", bufs=1))
                     for l in (0, 1)}
        sts = {}
        for lvl in (0, 1):
            sts[lvl] = _emit_level_p1(nc, tc, lvl_pools[lvl], lvl, s_sh,
                                      radios[lvl], b1t, aps, accs)
        for lvl in (0, 1):
            _emit_level_p2(nc, tc, lvl_pools[lvl], lvl, sts[lvl], aps, accs)

        # ---------- rgb ----------
        with tc.tile_pool(name="rgb", bufs=1) as pool:
            pdgt = pool.tile([P, NBLK * 6], dt.float32, tag="pdgt")
            SP.dma_start(_blk(pdgt[:], 6),
                         aps["pdgt"].rearrange("(b p) c -> p b c", p=P))
            pg3 = _blk(pdgt[:], 6)
            d = pool.tile([P, NBLK * 3], dt.float32, tag="rgbd")
            VE.tensor_tensor(_blk(d[:], 3), pg3[:, :, 0:3], pg3[:, :, 3:6],
                             Alu.subtract)
            dsq = pool.tile([P, NBLK * 3], dt.float32, tag="rgbsq")
            ACT.activation(dsq[:], d[:], ACTF.Square, accum_out=accs["rgb"][:])


        # ---------- hash loss (emitted first: fills the DMA warmup gap) ----
        ones_h = cpool.tile([P, HCOLS], dt.float32, tag="ones_h")
        PL.memset(ones_h[:], 1.0)
        for lvl in (0, 1):
            with tc.tile_pool(name=f"hash{lvl}", bufs=1) as pool:
                idx = pool.tile([P, HCOLS], dt.uint16, tag="hidx")
                src = aps[f"hi{lvl}"]
                SP.dma_start(idx[:], bass.AP(tensor=src.tensor,
                                             offset=src.offset,
                                             ap=[[HROW, P], [1, HCOLS]]))
                emb = pool.tile([P, HCOLS * 2], dt.float32, tag="hemb")
                esrc = aps[f"he{lvl}"]
                SP.dma_start(emb[:], bass.AP(tensor=esrc.tensor,
                                             offset=esrc.offset,
                                             ap=[[HROW * 2, P], [1, HCOLS * 2]]))
                sq = pool.tile([P, HCOLS * 2], dt.float32, tag="hsq")
                ACT.activation(sq[:], emb[:], ACTF.Square)
                wv = pool.tile([P, HCOLS], dt.float32, tag="hw")
                sq3 = sq[:].rearrange("p (n two) -> p n two", two=2)
                VE.tensor_tensor(wv[:], sq3[:, :, 0], sq3[:, :, 1], Alu.add)
                eq = pool.tile([P, HCOLS], dt.float32, tag="heq")
                PL.memset(eq[:, 0:1], 0.0)
                VE.tensor_tensor(eq[:, 1:HCOLS], idx[:, 1:HCOLS],
                                 idx[:, 0:HCOLS - 1], Alu.is_equal)
                S = pool.tile([P, HCOLS], dt.float32, tag="hS")
                PL.tensor_tensor_scan(S[:], eq[:], wv[:], 0.0, Alu.mult, Alu.add)
                cc = pool.tile([P, HCOLS], dt.float32, tag="hcc")
                PL.tensor_tensor_scan(cc[:], eq[:], ones_h[:], 0.0,
                                      Alu.mult, Alu.add)
                cci = pool.tile([P, HCOLS], dt.float32, tag="hcci")
                VE.reciprocal(cci[:], cc[:])
                ratio = pool.tile([P, HCOLS], dt.float32, tag="hr")
                VE.tensor_tensor(ratio[:], S[:], cci[:], Alu.mult)
                me = pool.tile([P, HCOLS], dt.float32, tag="hme")
                VE.tensor_scalar(me[:, 0:HCOLS - 1], eq[:, 1:HCOLS], -1.0, 1.0,
                                 Alu.mult, Alu.add)
                ttro = pool.tile([P, HROW], dt.float32, tag="httro")
                VE.tensor_tensor_reduce(ttro[:], ratio[:, HALO:HALO + HROW],
                                        me[:, HALO:HALO + HROW], 1.0, 0.0,
                                        Alu.mult, Alu.add,
                                        accs["hash" if lvl == 0 else "hash1"][:])


        # ---------- combine + output ----------
        with tc.tile_pool(name="fin", bufs=1) as pool:
            tot = pool.tile([P, 1], dt.float32, tag="tot")
            VE.tensor_scalar(tot[:], accs["rgb"][:], W_RGB / (R * 3), None,
                             Alu.mult)
            VE.scalar_tensor_tensor(tot[:], accs["inter"][:], W_INTER,
                                    tot[:], Alu.mult, Alu.add)
            VE.scalar_tensor_tensor(tot[:], accs["inter1"][:], W_INTER,
                                    tot[:], Alu.mult, Alu.add)
            VE.scalar_tensor_tensor(tot[:], accs["p1"][:], W_DIST / R,
                                    tot[:], Alu.mult, Alu.add)
            VE.scalar_tensor_tensor(tot[:], accs["p2"][:], W_DIST / (3.0 * R),
                                    tot[:], Alu.mult, Alu.add)
            VE.scalar_tensor_tensor(tot[:], accs["hash"][:],
                                    W_HASH / (NUM_SEGMENTS * 2.0), tot[:],
                                    Alu.mult, Alu.add)
            VE.scalar_tensor_tensor(tot[:], accs["hash1"][:],
                                    W_HASH / (NUM_SEGMENTS * 2.0), tot[:],
                                    Alu.mult, Alu.add)
            res = pool.tile([P, 1], dt.float32, tag="res")
            PL.partition_all_reduce(res[:], tot[:], channels=P,
                                    reduce_op=bass_isa.ReduceOp.add)
            SP.dma_start(out_ap, res[0:1, 0:1])
            import os
            if os.environ.get("KDBG") and "dbg" in aps:
                dbg = pool.tile([P, 7], dt.float32, tag="dbg")
                for i, name in enumerate(("rgb", "inter", "inter1", "p1",
                                          "p2", "hash", "hash1")):
                    VE.tensor_copy(dbg[:, i:i + 1], accs[name][:])
                SP.dma_start(aps["dbg"], dbg[:])


# ---------------- host side ----------------
_module_cache = {}


def _get_module():
    if "nc" not in _module_cache:
        _module_cache["nc"] = build_module()
    return _module_cache["nc"]


def shard_inputs(inputs):
    """Full inputs -> list of 8 per-core in_maps."""
    f32 = np.float32
    pd = np.ascontiguousarray(inputs["pd_rgbs"], f32)
    gt = np.ascontiguousarray(inputs["gt_rgbs"], f32)
    sd = np.ascontiguousarray(inputs["render_sdist"], f32)
    rw = np.ascontiguousarray(inputs["render_weights"], f32)
    ps0 = np.ascontiguousarray(inputs["prop_sdist_0"], f32)
    pw0 = np.ascontiguousarray(inputs["prop_weights_0"], f32)
    ps1 = np.ascontiguousarray(inputs["prop_sdist_1"], f32)
    pw1 = np.ascontiguousarray(inputs["prop_weights_1"], f32)
    hashes = {}
    for lvl in (0, 1):
        idx = np.asarray(inputs[f"enc_idx_{lvl}"]).astype(np.int64)
        emb = np.ascontiguousarray(inputs[f"enc_embds_{lvl}"], f32)
        idx_pad = np.empty(M + 2 * HALO, np.uint16)
        idx_pad[HALO:HALO + M] = idx.astype(np.uint16)
        # pads must differ from the adjacent real idx (run-break sentinels)
        idx_pad[:HALO] = np.uint16((int(idx[0]) + 1) & 0xFFFF)
        idx_pad[HALO + M:] = np.uint16((int(idx[-1]) + 1) & 0xFFFF)
        emb_pad = np.zeros((M + 2 * HALO, 2), f32)
        emb_pad[HALO:HALO + M] = emb
        hashes[lvl] = (idx_pad, emb_pad)

    consts = {}
    for lvl, L in LVL.items():
        LW, QWS, X = L["LW"], L["QWS"], L["X"]
        p1 = np.tile(np.arange(1, LW + 1, dtype=np.int16), NBLK)
        ic = np.concatenate([np.arange(1, LW + 1, dtype=np.int16) + b * QWS
                             for b in range(NBLK)])
        xl = np.concatenate([np.arange(X, dtype=np.int16) + b * LW
                             for b in range(NBLK)])
        row = np.concatenate([ic, xl])
        consts[f"c_iota_l{lvl}"] = np.ascontiguousarray(np.tile(row, (P, 1)))

    pdgt = np.concatenate([pd, gt], axis=1)
    sdrw = np.concatenate([sd, rw], axis=1)
    pspw = {0: np.concatenate([ps0, pw0], axis=1),
            1: np.concatenate([ps1, pw1], axis=1)}
    pslh = {}
    for lvl, ps in ((0, ps0), (1, ps1)):
        pu = ps.view(np.uint16).reshape(R, -1, 2)
        pslh[lvl] = (np.ascontiguousarray(pu[:, :, 0]),
                     np.ascontiguousarray(pu[:, :, 1]))

    in_maps = []
    for c in range(N_CORES):
        r0 = c * RPC
        lo = c * MPC
        im = {
            "pdgt": pdgt[r0:r0 + RPC],
            "sdrw": sdrw[r0:r0 + RPC],
            "pspw0": pspw[0][r0:r0 + RPC], "pspw1": pspw[1][r0:r0 + RPC],
            "pslo0": pslh[0][0][r0:r0 + RPC], "pshi0": pslh[0][1][r0:r0 + RPC],
            "pslo1": pslh[1][0][r0:r0 + RPC], "pshi1": pslh[1][1][r0:r0 + RPC],
        }
        for lvl in (0, 1):
            idx_pad, emb_pad = hashes[lvl]
            im[f"hi{lvl}"] = np.ascontiguousarray(idx_pad[lo:lo + HSLICE])
            im[f"he{lvl}"] = np.ascontiguousarray(
                emb_pad[lo:lo + HSLICE].reshape(-1))
        im.update(consts)
        in_maps.append(im)
    return in_maps


def kernel(**inputs) -> np.ndarray:
    nc = _get_module()
    in_maps = shard_inputs(inputs)
    res = run_bass_kernel_spmd(nc, in_maps, core_ids=list(range(N_CORES)))
    total = np.float64(0.0)
    for r in res.results:
        total += np.float64(r["out"][0, 0])
    return np.float32(total)


# revision 42
# speedup vs baseline: 1.1798x; 1.0517x over previous
"""Trainium2 Bass kernel for nn_Loss_dict_50646254354805 (NeRF-style loss).

Self-contained: accepts FULL inputs, shards across 8 NeuronCores (rays for
the per-ray losses, samples for the hash loss), runs one SPMD Bass module,
host-sums the 8 partial scalars.

Inter-loss: the reference's blur_step_function + sorted_interp_quad is
evaluated in a merged domain. Keys are uint16 quantized values (14-bit grid)
with 2-bit source tags, bitonic-merged at 2x DVE rate; per-slot values come
from the keys (grid error ~6e-5, validated ~1e-2 rel on the inter terms,
~0.5% on the total loss vs 2e-2 budget); the +-radio slopes are scattered
as exact f32 halves (their telescoping cancellation needs full precision).
Density/CDF reconstruction runs as masked prefix scans on the Pool engine;
conversions/relu/square run on the Activation engine; counts, positions and
compaction indices are uint16 DVE ops at 2-4x rate.
"""
import numpy as np

import concourse.bass as bass
import concourse.bass_isa as bass_isa
import concourse.mybir as mybir
import concourse.tile as tile
from concourse import bacc
from concourse.bass_utils import run_bass_kernel_spmd

dt = mybir.dt
Alu = mybir.AluOpType
AX = mybir.AxisListType
ACTF = mybir.ActivationFunctionType
P = 128

# problem constants
PULSE = (0.01, 0.005)
W_RGB, W_INTER, W_DIST, W_HASH = 1.0, 1.0, 0.01, 0.1
NUM_SEGMENTS = 65536
R, N = 4096, 48
M = R * N
N_CORES = 8
RPC = R // N_CORES            # rays per core (512)
NBLK = RPC // P               # ray tiles per core (4)
MPC = M // N_CORES            # hash samples per core (24576)
HALO = 64                     # hash run halo
HROW = MPC // P               # hash samples per partition (192)
HCOLS = HROW + HALO + 1       # loaded cols per partition (257)
HSLICE = HALO + MPC + HALO    # per-core hash slice length (24704)

# key quantization: key = trunc((v + OFF) * S4), tags in the low 2 bits
S4 = 63000.0
OFF = 0.02
PADK = 0xFFFC                 # pad key (tag 0, larger than any real key)

# per-level geometry
LVL = {0: dict(X=257, n2=512), 1: dict(X=97, n2=256)}
for _L in LVL.values():
    _L["EW"] = ((_L["X"] + 98 + 1 + 7) // 8) * 8        # 360 / 200
    _L["LW"] = _L["EW"] + 24                            # 384 / 224
    _L["NL"] = NBLK * _L["LW"]                          # 1536 / 896
    _L["SL"] = NBLK * _L["n2"]                          # 2048 / 1024
    _L["NW"] = NBLK * (_L["X"] - 1)                     # 1024 / 384
    _L["QWS"] = _L["LW"] - 98                           # 286 / 126
    _L["NQ"] = NBLK * _L["QWS"]


def _ts_int(eng, out, in0, imm1, op0, imm2=None, op1=None):
    """tensor_scalar with int32 immediates (for bitwise/compare ops)."""
    ins_ = [eng.lower_ap(in0), mybir.ImmediateValue(dtype=dt.int32, value=int(imm1))]
    kw = dict(op0=op0)
    if imm2 is not None:
        ins_.append(mybir.ImmediateValue(dtype=dt.int32, value=int(imm2)))
        kw["op1"] = op1
    return eng.add_instruction(mybir.InstTensorScalarPtr(
        name=eng.bass.get_next_instruction_name(),
        ins=ins_, outs=[eng.lower_ap(out)], **kw))


def _blk(ap, n2):
    return ap.rearrange("p (b n) -> p b n", b=NBLK)


def _merge_stages(VE, bufa, bufb, width, d_list, descending=False):
    """Full bitonic merge stages (ping-pong) over [P, G*width] u16 tiles."""
    cur, nxt = bufa, bufb
    for d in d_list:
        c3 = cur[:].rearrange("p (c td) -> p c td", td=2 * d)
        n3 = nxt[:].rearrange("p (c td) -> p c td", td=2 * d)
        lo_in, hi_in = c3[:, :, 0:d], c3[:, :, d:2 * d]
        if descending:
            VE.tensor_tensor(n3[:, :, 0:d], lo_in, hi_in, Alu.max)
            VE.tensor_tensor(n3[:, :, d:2 * d], lo_in, hi_in, Alu.min)
        else:
            VE.tensor_tensor(n3[:, :, 0:d], lo_in, hi_in, Alu.min)
            VE.tensor_tensor(n3[:, :, d:2 * d], lo_in, hi_in, Alu.max)
        cur, nxt = nxt, cur
    return cur, nxt


def _emit_level_p1(nc, tc, pool, lvl, s_sh, radio, b1t, aps, accs):
    """Phase 1: merge, flags/counts, scatters, exact values, radio."""
    VE, PL, ACT, SP = nc.vector, nc.gpsimd, nc.scalar, nc.sync
    L = LVL[lvl]
    X, n2, EW, LW, NL, SL, NW, QWS, NQ = (L["X"], L["n2"], L["EW"], L["LW"],
                                          L["NL"], L["SL"], L["NW"], L["QWS"],
                                          L["NQ"])
    pw = PULSE[lvl]

    def blkL(ap):
        return ap.rearrange("p (b n) -> p b n", b=NBLK)

    st = dict(blkL=blkL)

    # ---------- per-level constants (one batched DMA on SP) ----------
    iotas = pool.tile([P, NL + NBLK * X], dt.int16, tag="iotas")
    SP.dma_start(iotas[:], aps[f"c_iota_l{lvl}"][:, 0:NL + NBLK * X])
    iotaC = iotas[:][:, 0:NL]
    aps_iotaxl = iotas[:][:, NL:]
    maskf = pool.tile([P, NL], dt.float32, tag="maskf")
    PL.memset(maskf[:], 1.0)
    PL.memset(blkL(maskf[:])[:, :, 0:1], 0.0)
    st["maskf"] = maskf

    # ---------- inputs (one batched DMA: [ps | pw] per ray) ----------
    pspw = pool.tile([P, NBLK * (2 * X - 1)], dt.float32, tag="pspw")
    SP.dma_start(_blk(pspw[:], 2 * X - 1),
                 aps[f"pspw{lvl}"].rearrange("(b p) x -> p b x", p=P))
    xt3 = _blk(pspw[:], 2 * X - 1)[:, :, 0:X]
    pwt3 = _blk(pspw[:], 2 * X - 1)[:, :, X:2 * X - 1]
    st["pwt3"] = pwt3
    dinv = pool.tile([P, NW], dt.float32, tag="dinv")
    ACT.activation(_blk(dinv[:], X - 1), pwt3, ACTF.Copy, bias=1e-5)
    VE.reciprocal(dinv[:], dinv[:])
    st["dinv"] = dinv

    # ---------- big merge: queries + events (from b1t), ascending ----------
    B0 = pool.tile([P, SL], dt.uint16, tag="big0")
    B1 = pool.tile([P, SL], dt.uint16, tag="big1")
    b03 = _blk(B0[:], n2)
    PL.memset(b03[:, :, X:n2 - 128], PADK)
    # quantized query keys written straight into the merge buffer
    ACT.activation(b03[:, :, 0:X], xt3, ACTF.Copy, scale=S4, bias=OFF * S4)
    _ts_int(VE, b03[:, :, 0:X], b03[:, :, 0:X], 0xFFFC, Alu.bitwise_and)
    b1f = b1t[:].rearrange("p (g n) -> p g n", n=128)
    g0 = lvl * NBLK
    VE.tensor_copy(b03[:, :, n2 - 128:n2], b1f[:, g0:g0 + NBLK, ::-1])
    # first stage: only the trailing 98 pairs touch real data
    d0 = n2 // 2
    VE.tensor_tensor(_blk(B1[:], n2)[:, :, d0 - 98:d0],
                     b03[:, :, d0 - 98:d0], b03[:, :, n2 - 98:n2], Alu.min)
    VE.tensor_tensor(_blk(B1[:], n2)[:, :, n2 - 98:n2],
                     b03[:, :, d0 - 98:d0], b03[:, :, n2 - 98:n2], Alu.max)
    VE.tensor_copy(_blk(B1[:], n2)[:, :, 0:d0 - 98], b03[:, :, 0:d0 - 98])
    VE.tensor_copy(_blk(B1[:], n2)[:, :, d0:n2 - 98], b03[:, :, d0:n2 - 98])
    ds_rest = [n2 // 4]
    while ds_rest[-1] > 1:
        ds_rest.append(ds_rest[-1] // 2)
    Kt, Ksc = _merge_stages(VE, B1, B0, n2, ds_rest)
    mS = _blk(Kt[:], n2)[:, :, 0:LW]       # merged keys, strided [P,NBLK,LW]
    st["mS"] = mS
    st["Ksc"] = Ksc

    # ---------- flags / counts (u16) ----------
    ev16 = pool.tile([P, NL], dt.uint16, tag="ev16")
    _ts_int(VE, blkL(ev16[:]), mS, 1, Alu.bitwise_and)
    em16 = pool.tile([P, NL], dt.uint16, tag="em16")
    _ts_int(VE, blkL(em16[:]), mS, 3, Alu.bitwise_and, 1, Alu.is_equal)
    ep16 = pool.tile([P, NL], dt.uint16, tag="ep16")
    _ts_int(VE, blkL(ep16[:]), mS, 3, Alu.bitwise_and, 3, Alu.is_equal)
    HNL = NL // 2
    C16 = pool.tile([P, NL], dt.uint16, tag="C16")
    VE.tensor_tensor_scan(C16[:, 0:HNL], maskf[:, 0:HNL], ev16[:, 0:HNL],
                          0.0, Alu.mult, Alu.add)
    PL.tensor_tensor_scan(C16[:, HNL:NL], maskf[:, HNL:NL], ev16[:, HNL:NL],
                          0.0, Alu.mult, Alu.add)
    Cm16 = pool.tile([P, NL], dt.uint16, tag="Cm16")
    VE.tensor_tensor_scan(Cm16[:, 0:HNL], maskf[:, 0:HNL], em16[:, 0:HNL],
                          0.0, Alu.mult, Alu.add)
    PL.tensor_tensor_scan(Cm16[:, HNL:NL], maskf[:, HNL:NL], em16[:, HNL:NL],
                          0.0, Alu.mult, Alu.add)
    st["ev16"] = ev16
    st["ep16"] = ep16

    # ---------- event position scatters ----------
    tmp16 = pool.tile([P, NL], dt.uint16, tag="tmp16")
    idx16 = pool.tile([P, NL], dt.int16, tag="idx16")
    t3 = blkL(tmp16[:])
    i3 = blkL(idx16[:])
    C3, Cm3, em3, ep3 = (blkL(C16[:]), blkL(Cm16[:]), blkL(em16[:]),
                         blkL(ep16[:]))
    pos_m = pool.tile([P, NBLK * 64], dt.uint16, tag="pos_m")
    pos_p = pool.tile([P, NBLK * 64], dt.uint16, tag="pos_p")
    for which, pos in ((0, pos_m), (1, pos_p)):
        if which == 0:
            VE.tensor_tensor(t3[:, :, 0:EW], Cm3[:, :, 0:EW], em3[:, :, 0:EW],
                             Alu.mult)
        else:
            VE.tensor_tensor(t3[:, :, 0:EW], C3[:, :, 0:EW], Cm3[:, :, 0:EW],
                             Alu.subtract)
            VE.tensor_tensor(t3[:, :, 0:EW], t3[:, :, 0:EW], ep3[:, :, 0:EW],
                             Alu.mult)
        _ts_int(VE, i3[:, :, 0:EW], t3[:, :, 0:EW], -1, Alu.add)
        for b in range(NBLK):
            PL.local_scatter(pos[:, b * 64:(b + 1) * 64],
                             iotaC[:, b * LW:b * LW + EW].bitcast(dt.uint16),
                             idx16[:, b * LW:b * LW + EW], channels=P,
                             num_elems=64, num_idxs=EW)

    # ---------- radio scatter (exact f32 halves) ----------
    CW = NBLK * 128 + NBLK * X
    idxcat = pool.tile([P, CW], dt.int16, tag="idxcat")
    tgt16 = idxcat[:][:, 0:NBLK * 128]
    tg3 = _blk(tgt16, 128)
    pm3 = _blk(pos_m[:], 64)
    pp3 = _blk(pos_p[:], 64)
    for b in range(NBLK):
        # iotaC data carries +b*QWS; fold its removal into the block offset
        _ts_int(VE, tg3[:, b, 0:49], pm3[:, b, 0:49], b * (LW - QWS) - 1, Alu.add)
        _ts_int(VE, tg3[:, b, 49:98], pp3[:, b, 0:49], b * (LW - QWS) - 1, Alu.add)
    PL.memset(tg3[:, :, 98:128], -1)

    radcat = pool.tile([P, NBLK * 128], dt.float32, tag="radcat")
    r3 = _blk(radcat[:], 128)
    VE.tensor_copy(r3[:, :, 0:49], _blk(radio[:], 49))
    VE.tensor_scalar(r3[:, :, 49:98], _blk(radio[:], 49), -1.0, None, Alu.mult)
    PL.memset(r3[:, :, 98:128], 0.0)
    rc_u = radcat[:].bitcast(dt.uint16).rearrange("p (n two) -> p n two", two=2)
    rad_lo = pool.tile([P, NBLK * 128], dt.uint16, tag="rad_lo")
    rad_hi = pool.tile([P, NBLK * 128], dt.uint16, tag="rad_hi")
    VE.tensor_copy(rad_lo[:], rc_u[:, :, 0])
    VE.tensor_copy(rad_hi[:], rc_u[:, :, 1])
    rl_t = pool.tile([P, NL], dt.uint16, tag="rl_t")
    rh_t = pool.tile([P, NL], dt.uint16, tag="rh_t")
    PL.local_scatter(rl_t[:], rad_lo[:], tgt16, channels=P,
                     num_elems=NL, num_idxs=NBLK * 128)
    PL.local_scatter(rh_t[:], rad_hi[:], tgt16, channels=P,
                     num_elems=NL, num_idxs=NBLK * 128)
    radio_m = pool.tile([P, NL], dt.float32, tag="radio_m")
    rm_u = radio_m[:].bitcast(dt.uint16).rearrange("p (n two) -> p n two", two=2)
    ACT.activation(rm_u[:, :, 0], rl_t[:], ACTF.Copy)
    ACT.activation(rm_u[:, :, 1], rh_t[:], ACTF.Copy)
    st["radio_m"] = radio_m

    # ---------- compaction indices (reused later for the cdf compact) ------
    qf16 = em16                           # em16 dead after pos idx
    _ts_int(VE, blkL(qf16[:]), mS, 3, Alu.bitwise_and, 0, Alu.is_equal)
    tq = tmp16                            # tmp16 dead after pos idx
    VE.tensor_tensor(tq[:], iotaC.bitcast(dt.uint16), C16[:], Alu.subtract)
    VE.tensor_tensor(tq[:], tq[:], qf16[:], Alu.mult)
    idxq = pool.tile([P, NL], dt.int16, tag="idxq")
    _ts_int(VE, idxq[:], tq[:], -1, Alu.add)
    st["idxq"] = idxq

    # ---------- exact per-slot values (queries + events, one scatter) ------
    i0q = qf16                            # qf16 dead after idxq
    PL.local_scatter(i0q[:, 0:NQ], C16[:], idxq[:], channels=P,
                     num_elems=NQ, num_idxs=NL)
    VE.tensor_tensor(_blk(idxcat[:][:, NBLK * 128:CW], X).bitcast(dt.uint16),
                     aps_iotaxl.bitcast(dt.uint16).rearrange(
                         "p (b n) -> p b n", b=NBLK),
                     _blk(i0q[:, 0:NQ], QWS)[:, :, 0:X], Alu.add)
    emsh = pool.tile([P, NBLK * 49], dt.float32, tag="emsh")
    ACT.activation(_blk(emsh[:], 49), s_sh, ACTF.Copy, bias=-pw)
    epsh = pool.tile([P, NBLK * 49], dt.float32, tag="epsh")
    ACT.activation(_blk(epsh[:], 49), s_sh, ACTF.Copy, bias=pw)
    vc_lo = pool.tile([P, CW], dt.uint16, tag="vc_lo")
    vc_hi = pool.tile([P, CW], dt.uint16, tag="vc_hi")
    em_u = emsh[:].bitcast(dt.uint16).rearrange("p (b n two) -> p b n two",
                                                b=NBLK, two=2)
    ep_u = epsh[:].bitcast(dt.uint16).rearrange("p (b n two) -> p b n two",
                                                b=NBLK, two=2)
    for half, vc, hname in ((0, vc_lo, "pslo"), (1, vc_hi, "pshi")):
        vch = _blk(vc[:][:, 0:NBLK * 128], 128)
        VE.tensor_copy(vch[:, :, 0:49], em_u[:, :, :, half])
        VE.tensor_copy(vch[:, :, 49:98], ep_u[:, :, :, half])
        PL.memset(vch[:, :, 98:128], 0)
        SP.dma_start(_blk(vc[:][:, NBLK * 128:CW], X),
                     aps[f"{hname}{lvl}"].rearrange("(b p) x -> p b x", p=P))
    vl_t = pool.tile([P, NL], dt.uint16, tag="vl_t")
    vh_t = pool.tile([P, NL], dt.uint16, tag="vh_t")
    PL.local_scatter(vl_t[:], vc_lo[:], idxcat[:], channels=P,
                     num_elems=NL, num_idxs=CW)
    PL.local_scatter(vh_t[:], vc_hi[:], idxcat[:], channels=P,
                     num_elems=NL, num_idxs=CW)
    v = pool.tile([P, NL], dt.float32, tag="v")
    v_u = v[:].bitcast(dt.uint16).rearrange("p (n two) -> p n two", two=2)
    ACT.activation(v_u[:, :, 0], vl_t[:], ACTF.Copy)
    ACT.activation(v_u[:, :, 1], vh_t[:], ACTF.Copy)
    dv = pool.tile([P, NL], dt.float32, tag="dv")
    dv3 = blkL(dv[:])
    v3 = blkL(v[:])
    VE.tensor_tensor(dv3[:, 0:2, 1:EW], v3[:, 0:2, 1:EW], v3[:, 0:2, 0:EW - 1],
                     Alu.subtract)
    PL.tensor_tensor(dv3[:, 2:4, 1:EW], v3[:, 2:4, 1:EW], v3[:, 2:4, 0:EW - 1],
                     Alu.subtract)
    st["dv"] = dv
    st["v"] = v
    st["vl_t"] = vl_t
    st["vh_t"] = vh_t
    return st


def _emit_level_p2(nc, tc, pool, lvl, st, aps, accs):
    """Phase 2: density chain, cdf compaction, loss tail."""
    VE, PL, ACT, SP = nc.vector, nc.gpsimd, nc.scalar, nc.sync
    L = LVL[lvl]
    X, EW, LW, NL, NW, QWS, NQ = (L["X"], L["EW"], L["LW"], L["NL"], L["NW"],
                                  L["QWS"], L["NQ"])
    blkL = st["blkL"]
    maskf, radio_m, dv, dinv = st["maskf"], st["radio_m"], st["dv"], st["dinv"]
    idxq, Ksc = st["idxq"], st["Ksc"]
    dv3 = blkL(dv[:])

    # ---------- density chain, block-split: blocks 0-1 on DVE, 2-3 on
    # Pool; halves run concurrently, halving the serial chain ----------
    HNL = NL // 2
    h0 = slice(0, 2)
    h1 = slice(2, 4)
    g = pool.tile([P, NL], dt.float32, tag="g")
    VE.tensor_tensor_scan(g[:, 0:HNL], maskf[:, 0:HNL], radio_m[:, 0:HNL],
                          0.0, Alu.mult, Alu.add)
    PL.tensor_tensor_scan(g[:, HNL:NL], maskf[:, HNL:NL], radio_m[:, HNL:NL],
                          0.0, Alu.mult, Alu.add)
    wg = radio_m                          # radio_m dead after g scan
    wg3 = blkL(wg[:])
    PL.memset(wg3[:, :, 0:1], 0.0)
    PL.memset(wg3[:, :, EW:LW], 0.0)
    g3 = blkL(g[:])
    VE.tensor_tensor(wg3[:, h0, 1:EW], dv3[:, h0, 1:EW], g3[:, h0, 0:EW - 1],
                     Alu.mult)
    PL.tensor_tensor(wg3[:, h1, 1:EW], dv3[:, h1, 1:EW], g3[:, h1, 0:EW - 1],
                     Alu.mult)
    w = pool.tile([P, NL], dt.float32, tag="w")
    VE.tensor_tensor_scan(w[:, 0:HNL], maskf[:, 0:HNL], wg[:, 0:HNL],
                          0.0, Alu.mult, Alu.add)
    PL.tensor_tensor_scan(w[:, HNL:NL], maskf[:, HNL:NL], wg[:, HNL:NL],
                          0.0, Alu.mult, Alu.add)
    wc = w                                # relu in place
    VE.tensor_scalar(wc[:, 0:HNL], w[:, 0:HNL], 0.0, None, Alu.max)
    ACT.activation(wc[:, HNL:NL], w[:, HNL:NL], ACTF.Relu)
    scr = g                               # g dead after wg
    scr3 = blkL(scr[:])
    wc3 = blkL(wc[:])
    VE.tensor_tensor(scr3[:, h0, 1:EW], wc3[:, h0, 1:EW], wc3[:, h0, 0:EW - 1],
                     Alu.add)
    PL.tensor_tensor(scr3[:, h1, 1:EW], wc3[:, h1, 1:EW], wc3[:, h1, 0:EW - 1],
                     Alu.add)
    area = wg                             # wg dead after w scan
    a3 = blkL(area[:])
    PL.memset(a3[:, :, 0:1], 0.0)
    PL.memset(a3[:, :, EW:LW], 0.0)
    # 0.5 of the trapezoid is pre-folded into the radio scale (1/(4*pw))
    VE.tensor_tensor(a3[:, h0, 1:EW], scr3[:, h0, 1:EW], dv3[:, h0, 1:EW],
                     Alu.mult)
    PL.tensor_tensor(a3[:, h1, 1:EW], scr3[:, h1, 1:EW], dv3[:, h1, 1:EW],
                     Alu.mult)
    cdf = dv                              # dv dead after area
    VE.tensor_tensor_scan(cdf[:, 0:HNL], maskf[:, 0:HNL], area[:, 0:HNL],
                          0.0, Alu.mult, Alu.add)
    PL.tensor_tensor_scan(cdf[:, HNL:NL], maskf[:, HNL:NL], area[:, HNL:NL],
                          0.0, Alu.mult, Alu.add)

    # ---------- compact cdf at query slots ----------
    cdf_lo = st["ev16"]                   # dead after C scan
    cdf_hi = st["ep16"]                   # dead after pos idx
    cdf_u = cdf[:].bitcast(dt.uint16).rearrange("p (n two) -> p n two", two=2)
    VE.tensor_copy(cdf_lo[:], cdf_u[:, :, 0])
    ACT.activation(cdf_hi[:], cdf_u[:, :, 1], ACTF.Copy)
    cq_lo = st["vl_t"]                    # dead after v recombine
    cq_hi = st["vh_t"]
    PL.local_scatter(cq_lo[:, 0:NQ], cdf_lo[:], idxq[:], channels=P,
                     num_elems=NQ, num_idxs=NL)
    PL.local_scatter(cq_hi[:, 0:NQ], cdf_hi[:], idxq[:], channels=P,
                     num_elems=NQ, num_idxs=NL)
    cdfq = pool.tile([P, NBLK * X], dt.float32, tag="cdfq")
    cq_u = cdfq[:].bitcast(dt.uint16).rearrange("p (b n two) -> p b n two",
                                                b=NBLK, two=2)
    VE.tensor_copy(cq_u[:, :, 0:X, 0], _blk(cq_lo[:, 0:NQ], QWS)[:, :, 0:X])
    ACT.activation(cq_u[:, :, 0:X, 1], _blk(cq_hi[:, 0:NQ], QWS)[:, :, 0:X],
                   ACTF.Copy)

    # ---------- loss tail (block-split DVE/Pool; relu on Act) ----------
    ws = cdf[:][:, 0:NW]                  # cdf dead after split
    cqf = _blk(cdfq[:], X)
    ws3 = ws.rearrange("p (b n) -> p b n", b=NBLK)
    VE.tensor_tensor(ws3[:, h0], cqf[:, h0, 1:X], cqf[:, h0, 0:X - 1],
                     Alu.subtract)
    PL.tensor_tensor(ws3[:, h1], cqf[:, h1, 1:X], cqf[:, h1, 0:X - 1],
                     Alu.subtract)
    t = wc[:][:, 0:NW]                    # wc dead after scr
    t3v = t.rearrange("p (b n) -> p b n", b=NBLK)
    VE.tensor_tensor(t3v[:, h0], ws3[:, h0], st["pwt3"][:, h0], Alu.subtract)
    PL.tensor_tensor(t3v[:, h1], ws3[:, h1], st["pwt3"][:, h1], Alu.subtract)
    r = scr[:][:, 0:NW]                   # scr dead after area
    ACT.activation(r, t, ACTF.Relu)       # concurrent with u on DVE/Pool
    u = area[:][:, 0:NW]                  # area dead after cdf scan
    HNW = NW // 2
    VE.tensor_tensor(u[:, 0:HNW], t[:, 0:HNW], dinv[:][:, 0:HNW], Alu.mult)
    PL.tensor_tensor(u[:, HNW:NW], t[:, HNW:NW], dinv[:][:, HNW:NW], Alu.mult)
    ttro = Ksc[:].bitcast(dt.float32)[:, 0:NW]   # merge scratch, long dead
    VE.tensor_tensor_reduce(ttro, u, r, 1.0 / (R * (X - 1)), 0.0,
                            Alu.mult, Alu.add,
                            accs["inter" if lvl == 0 else "inter1"][:])


def build_module():
    nc = bacc.Bacc("TRN2", target_bir_lowering=False, debug=False,
                   enable_asserts=False, num_devices=N_CORES)
    aps = {}

    def din(name, shape, dtype=dt.float32):
        aps[name] = nc.dram_tensor(name, shape, dtype, kind="ExternalInput").ap()
    din("pdgt", [RPC, 6])
    din("sdrw", [RPC, 97])
    din("pspw0", [RPC, 513]); din("pspw1", [RPC, 193])
    din("pslo0", [RPC, 257], dt.uint16); din("pshi0", [RPC, 257], dt.uint16)
    din("pslo1", [RPC, 97], dt.uint16); din("pshi1", [RPC, 97], dt.uint16)
    din("hi0", [HSLICE], dt.uint16); din("he0", [HSLICE * 2])
    din("hi1", [HSLICE], dt.uint16); din("he1", [HSLICE * 2])
    for lvl in (0, 1):
        nl = LVL[lvl]["NL"]
        din(f"c_iota_l{lvl}", [P, nl + NBLK * LVL[lvl]["X"]], dt.int16)
    out_ap = nc.dram_tensor("out", [1, 1], dt.float32, kind="ExternalOutput").ap()
    import os
    if os.environ.get("KDBG"):
        aps["dbg"] = nc.dram_tensor("dbg", [P, 7], dt.float32,
                                    kind="ExternalOutput").ap()
        for lvl in (0, 1):
            L = LVL[lvl]
            aps[f"dbgk{lvl}"] = nc.dram_tensor(f"dbgk{lvl}", [P, L["SL"]],
                                               dt.uint16, kind="ExternalOutput").ap()
            aps[f"dbgr{lvl}"] = nc.dram_tensor(f"dbgr{lvl}", [P, L["NL"]],
                                               dt.uint16, kind="ExternalOutput").ap()
            aps[f"dbgc{lvl}"] = nc.dram_tensor(f"dbgc{lvl}", [P, NBLK * L["X"]],
                                               dt.float32, kind="ExternalOutput").ap()
            aps[f"dbgw{lvl}"] = nc.dram_tensor(f"dbgw{lvl}", [P, L["NW"]],
                                               dt.float32, kind="ExternalOutput").ap()

    with tile.TileContext(nc) as tc:
        _emit(nc, tc, aps, out_ap)
    nc.compile()
    return nc


def _emit(nc, tc, aps, out_ap):
    import contextlib
    VE, PL, ACT, SP = nc.vector, nc.gpsimd, nc.scalar, nc.sync
    with contextlib.ExitStack() as ctx:
        cpool = ctx.enter_context(tc.tile_pool(name="consts", bufs=1))
        accs = {}
        for name in ("rgb", "inter", "inter1", "p1", "p2", "hash", "hash1"):
            a = cpool.tile([P, 1], dt.float32, tag=f"acc_{name}")
            accs[name] = a

        # ---------- shared render tables + radio + dist ----------
        spool = ctx.enter_context(tc.tile_pool(name="shared", bufs=1))
        sdrw = spool.tile([P, NBLK * 97], dt.float32, tag="sdrw")
        SP.dma_start(_blk(sdrw[:], 97),
                     aps["sdrw"].rearrange("(b p) x -> p b x", p=P))
        s_sh = _blk(sdrw[:], 97)[:, :, 0:49]
        radios = {0: spool.tile([P, NBLK * 49], dt.float32, tag="radio0",
                                name="radio0"),
                  1: spool.tile([P, NBLK * 49], dt.float32, tag="radio1",
                                name="radio1")}
        b1t = spool.tile([P, 2 * NBLK * 128], dt.uint16, tag="b1t")

        with tc.tile_pool(name="setup", bufs=1) as pool:
            rwv = _blk(sdrw[:], 97)[:, :, 49:97]
            s3 = s_sh
            rw_sh = pool.tile([P, NBLK * 48], dt.float32, tag="rw_sh")
            VE.tensor_copy(_blk(rw_sh[:], 48), rwv)
            ds = pool.tile([P, NBLK * 48], dt.float32, tag="ds")
            VE.tensor_tensor(_blk(ds[:], 48), s3[:, :, 1:49], s3[:, :, 0:48],
                             Alu.subtract)
            dsi = pool.tile([P, NBLK * 48], dt.float32, tag="dsi")
            ACT.activation(dsi[:], ds[:], ACTF.Copy, bias=1e-8)
            VE.reciprocal(dsi[:], dsi[:])
            wnorm = pool.tile([P, NBLK * 48], dt.float32, tag="wnorm")
            VE.tensor_tensor(wnorm[:], rw_sh[:], dsi[:], Alu.mult)
            wnp = pool.tile([P, NBLK * 50], dt.float32, tag="wnp")
            PL.memset(wnp[:], 0.0)
            VE.tensor_copy(_blk(wnp[:], 50)[:, :, 1:49], _blk(wnorm[:], 48))
            diff = pool.tile([P, NBLK * 49], dt.float32, tag="diff")
            wnp3 = _blk(wnp[:], 50)
            VE.tensor_tensor(_blk(diff[:], 49), wnp3[:, :, 1:50],
                             wnp3[:, :, 0:49], Alu.subtract)
            for lvl in (0, 1):
                # 1/(4*pw): includes the 0.5 of the trapezoid area
                VE.tensor_scalar(radios[lvl][:], diff[:], 1.0 / (4 * PULSE[lvl]),
                                 None, Alu.mult)

            # ---------- shared event merge (both levels, 128-wide asc) -----
            # build into b1b, partial d=64 stage into b1t, then six full
            # stages ping-pong back into b1t.
            b1b = pool.tile([P, 2 * NBLK * 128], dt.uint16, tag="b1b")
            b1g = b1b[:].rearrange("p (g n) -> p g n", n=128)
            PL.memset(b1g[:, :, 49:79], PADK)
            for lvl in (0, 1):
                pw = PULSE[lvl]
                kem = pool.tile([P, NBLK * 49], dt.uint16, tag=f"kem{lvl}")
                ACT.activation(_blk(kem[:], 49), s3, ACTF.Copy, scale=S4,
                               bias=(OFF - pw) * S4)
                _ts_int(VE, kem[:], kem[:], 0xFFFC, Alu.bitwise_and, 1,
                        Alu.bitwise_or)
                kep = pool.tile([P, NBLK * 49], dt.uint16, tag=f"kep{lvl}")
                ACT.activation(_blk(kep[:], 49), s3, ACTF.Copy, scale=S4,
                               bias=(OFF + pw) * S4)
                _ts_int(VE, kep[:], kep[:], 0xFFFC, Alu.bitwise_and, 3,
                        Alu.bitwise_or)
                g0 = lvl * NBLK
                VE.tensor_copy(b1g[:, g0:g0 + NBLK, 0:49], _blk(kem[:], 49))
                VE.tensor_copy(b1g[:, g0:g0 + NBLK, 79:128],
                               _blk(kep[:], 49)[:, :, ::-1])
            # partial first stage (d=64): only pairs (15..63, 79..127) matter
            b1n = b1t[:].rearrange("p (g n) -> p g n", n=128)
            VE.tensor_tensor(b1n[:, :, 15:64], b1g[:, :, 15:64],
                             b1g[:, :, 79:128], Alu.min)
            VE.tensor_tensor(b1n[:, :, 79:128], b1g[:, :, 15:64],
                             b1g[:, :, 79:128], Alu.max)
            VE.tensor_copy(b1n[:, :, 0:15], b1g[:, :, 0:15])
            VE.tensor_copy(b1n[:, :, 64:79], b1g[:, :, 64:79])
            res, _ = _merge_stages(VE, b1t, b1b, 128, [32, 16, 8, 4, 2, 1])
            assert res is b1t

            # ---------- distortion ----------
            mask48 = pool.tile([P, NBLK * 48], dt.float32, tag="mask48")
            PL.memset(mask48[:], 1.0)
            PL.memset(_blk(mask48[:], 48)[:, :, 0:1], 0.0)
            mid = pool.tile([P, NBLK * 48], dt.float32, tag="mid")
            VE.tensor_tensor(_blk(mid[:], 48), s3[:, :, 1:49], s3[:, :, 0:48],
                             Alu.add)   # 2*mid; the 0.5 folds into W_DIST
            wm = pool.tile([P, NBLK * 48], dt.float32, tag="wm")
            VE.tensor_tensor(wm[:], rw_sh[:], mid[:], Alu.mult)
            Cin = pool.tile([P, NBLK * 48], dt.float32, tag="Cin")
            PL.tensor_tensor_scan(Cin[:], mask48[:], rw_sh[:], 0.0,
                                  Alu.mult, Alu.add)
            Sin = pool.tile([P, NBLK * 48], dt.float32, tag="Sin")
            PL.tensor_tensor_scan(Sin[:], mask48[:], wm[:], 0.0,
                                  Alu.mult, Alu.add)
            A = pool.tile([P, NBLK * 47], dt.float32, tag="A47")
            m3 = _blk(mid[:], 48)
            c3 = _blk(Cin[:], 48)
            sw3 = _blk(Sin[:], 48)
            rw3 = _blk(rw_sh[:], 48)
            A3 = _blk(A[:], 47)
            VE.tensor_tensor(A3, m3[:, :, 1:48], c3[:, :, 0:47], Alu.mult)
            VE.tensor_tensor(A3, A3, sw3[:, :, 0:47], Alu.subtract)
            ttro = pool.tile([P, NBLK * 47], dt.float32, tag="dttro")
            VE.tensor_tensor_reduce(_blk(ttro[:], 47), A3, rw3[:, :, 1:48],
                                    1.0, 0.0, Alu.mult, Alu.add, accs["p1"][:],
                                    opt_aps=False)
            t2 = pool.tile([P, NBLK * 48], dt.float32, tag="t2d")
            VE.tensor_tensor(t2[:], rw_sh[:], rw_sh[:], Alu.mult)
            ttro2 = pool.tile([P, NBLK * 48], dt.float32, tag="dttro2")
            VE.tensor_tensor_reduce(ttro2[:], t2[:], ds[:], 1.0, 0.0,
                                    Alu.mult, Alu.add, accs["p2"][:])

        # ---------- inter loss (levels interleaved phase-wise) ----------
        lvl_pools = {l: ctx.enter_context(tc.tile_pool(name=f"lvl{l}", bufs=1))
                     for l in (0, 1)}
        sts = {}
        for lvl in (0, 1):
            sts[lvl] = _emit_level_p1(nc, tc, lvl_pools[lvl], lvl, s_sh,
                                      radios[lvl], b1t, aps, accs)
        for lvl in (0, 1):
            _emit_level_p2(nc, tc, lvl_pools[lvl], lvl, sts[lvl], aps, accs)

        # ---------- rgb ----------
        with tc.tile_pool(name="rgb", bufs=1) as pool:
            pdgt = pool.tile([P, NBLK * 6], dt.float32, tag="pdgt")
            SP.dma_start(_blk(pdgt[:], 6),
                         aps["pdgt"].rearrange("(b p) c -> p b c", p=P))
            pg3 = _blk(pdgt[:], 6)
            d = pool.tile([P, NBLK * 3], dt.float32, tag="rgbd")
            VE.tensor_tensor(_blk(d[:], 3), pg3[:, :, 0:3], pg3[:, :, 3:6],
                             Alu.subtract)
            dsq = pool.tile([P, NBLK * 3], dt.float32, tag="rgbsq")
            ACT.activation(dsq[:], d[:], ACTF.Square, accum_out=accs["rgb"][:])


        # ---------- hash loss (emitted first: fills the DMA warmup gap) ----
        ones_h = cpool.tile([P, HCOLS], dt.float32, tag="ones_h")
        PL.memset(ones_h[:], 1.0)
        for lvl in (0, 1):
            with tc.tile_pool(name=f"hash{lvl}", bufs=1) as pool:
                idx = pool.tile([P, HCOLS], dt.uint16, tag="hidx")
                src = aps[f"hi{lvl}"]
                SP.dma_start(idx[:], bass.AP(tensor=src.tensor,
                                             offset=src.offset,
                                             ap=[[HROW, P], [1, HCOLS]]))
                emb = pool.tile([P, HCOLS * 2], dt.float32, tag="hemb")
                esrc = aps[f"he{lvl}"]
                SP.dma_start(emb[:], bass.AP(tensor=esrc.tensor,
                                             offset=esrc.offset,
                                             ap=[[HROW * 2, P], [1, HCOLS * 2]]))
                sq = pool.tile([P, HCOLS * 2], dt.float32, tag="hsq")
                ACT.activation(sq[:], emb[:], ACTF.Square)
                wv = pool.tile([P, HCOLS], dt.float32, tag="hw")
                sq3 = sq[:].rearrange("p (n two) -> p n two", two=2)
                VE.tensor_tensor(wv[:], sq3[:, :, 0], sq3[:, :, 1], Alu.add)
                eq = pool.tile([P, HCOLS], dt.float32, tag="heq")
                PL.memset(eq[:, 0:1], 0.0)
                VE.tensor_tensor(eq[:, 1:HCOLS], idx[:, 1:HCOLS],
                                 idx[:, 0:HCOLS - 1], Alu.is_equal)
                S = pool.tile([P, HCOLS], dt.float32, tag="hS")
                PL.tensor_tensor_scan(S[:], eq[:], wv[:], 0.0, Alu.mult, Alu.add)
                cc = pool.tile([P, HCOLS], dt.float32, tag="hcc")
                PL.tensor_tensor_scan(cc[:], eq[:], ones_h[:], 0.0,
                                      Alu.mult, Alu.add)
                cci = pool.tile([P, HCOLS], dt.float32, tag="hcci")
                VE.reciprocal(cci[:], cc[:])
                ratio = pool.tile([P, HCOLS], dt.float32, tag="hr")
                VE.tensor_tensor(ratio[:], S[:], cci[:], Alu.mult)
                me = pool.tile([P, HCOLS], dt.float32, tag="hme")
                VE.tensor_scalar(me[:, 0:HCOLS - 1], eq[:, 1:HCOLS], -1.0, 1.0,
                                 Alu.mult, Alu.add)
                ttro = pool.tile([P, HROW], dt.float32, tag="httro")
                VE.tensor_tensor_reduce(ttro[:], ratio[:, HALO:HALO + HROW],
                                        me[:, HALO:HALO + HROW], 1.0, 0.0,
                                        Alu.mult, Alu.add,
                                        accs["hash" if lvl == 0 else "hash1"][:])


        # ---------- combine + output ----------
        with tc.tile_pool(name="fin", bufs=1) as pool:
            tot = pool.tile([P, 1], dt.float32, tag="tot")
            VE.tensor_scalar(tot[:], accs["rgb"][:], W_RGB / (R * 3), None,
                             Alu.mult)
            VE.scalar_tensor_tensor(tot[:], accs["inter"][:], W_INTER,
                                    tot[:], Alu.mult, Alu.add)
            VE.scalar_tensor_tensor(tot[:], accs["inter1"][:], W_INTER,
                                    tot[:], Alu.mult, Alu.add)
            VE.scalar_tensor_tensor(tot[:], accs["p1"][:], W_DIST / R,
                                    tot[:], Alu.mult, Alu.add)
            VE.scalar_tensor_tensor(tot[:], accs["p2"][:], W_DIST / (3.0 * R),
                                    tot[:], Alu.mult, Alu.add)
            VE.scalar_tensor_tensor(tot[:], accs["hash"][:],
                                    W_HASH / (NUM_SEGMENTS * 2.0), tot[:],
                                    Alu.mult, Alu.add)
            VE.scalar_tensor_tensor(tot[:], accs["hash1"][:],
                                    W_HASH / (NUM_SEGMENTS * 2.0), tot[:],
                                    Alu.mult, Alu.add)
            res = pool.tile([P, 1], dt.float32, tag="res")
            PL.partition_all_reduce(res[:], tot[:], channels=P,
                                    reduce_op=bass_isa.ReduceOp.add)
            SP.dma_start(out_ap, res[0:1, 0:1])
            import os
            if os.environ.get("KDBG") and "dbg" in aps:
                dbg = pool.tile([P, 7], dt.float32, tag="dbg")
                for i, name in enumerate(("rgb", "inter", "inter1", "p1",
                                          "p2", "hash", "hash1")):
                    VE.tensor_copy(dbg[:, i:i + 1], accs[name][:])
                SP.dma_start(aps["dbg"], dbg[:])


# ---------------- host side ----------------
_module_cache = {}


def _get_module():
    if "nc" not in _module_cache:
        _module_cache["nc"] = build_module()
    return _module_cache["nc"]


def shard_inputs(inputs):
    """Full inputs -> list of 8 per-core in_maps."""
    f32 = np.float32
    pd = np.ascontiguousarray(inputs["pd_rgbs"], f32)
    gt = np.ascontiguousarray(inputs["gt_rgbs"], f32)
    sd = np.ascontiguousarray(inputs["render_sdist"], f32)
    rw = np.ascontiguousarray(inputs["render_weights"], f32)
    ps0 = np.ascontiguousarray(inputs["prop_sdist_0"], f32)
    pw0 = np.ascontiguousarray(inputs["prop_weights_0"], f32)
    ps1 = np.ascontiguousarray(inputs["prop_sdist_1"], f32)
    pw1 = np.ascontiguousarray(inputs["prop_weights_1"], f32)
    hashes = {}
    for lvl in (0, 1):
        idx = np.asarray(inputs[f"enc_idx_{lvl}"]).astype(np.int64)
        emb = np.ascontiguousarray(inputs[f"enc_embds_{lvl}"], f32)
        idx_pad = np.empty(M + 2 * HALO, np.uint16)
        idx_pad[HALO:HALO + M] = idx.astype(np.uint16)
        # pads must differ from the adjacent real idx (run-break sentinels)
        idx_pad[:HALO] = np.uint16((int(idx[0]) + 1) & 0xFFFF)
        idx_pad[HALO + M:] = np.uint16((int(idx[-1]) + 1) & 0xFFFF)
        emb_pad = np.zeros((M + 2 * HALO, 2), f32)
        emb_pad[HALO:HALO + M] = emb
        hashes[lvl] = (idx_pad, emb_pad)

    consts = {}
    for lvl, L in LVL.items():
        LW, QWS, X = L["LW"], L["QWS"], L["X"]
        p1 = np.tile(np.arange(1, LW + 1, dtype=np.int16), NBLK)
        ic = np.concatenate([np.arange(1, LW + 1, dtype=np.int16) + b * QWS
                             for b in range(NBLK)])
        xl = np.concatenate([np.arange(X, dtype=np.int16) + b * LW
                             for b in range(NBLK)])
        row = np.concatenate([ic, xl])
        consts[f"c_iota_l{lvl}"] = np.ascontiguousarray(np.tile(row, (P, 1)))

    pdgt = np.concatenate([pd, gt], axis=1)
    sdrw = np.concatenate([sd, rw], axis=1)
    pspw = {0: np.concatenate([ps0, pw0], axis=1),
            1: np.concatenate([ps1, pw1], axis=1)}
    pslh = {}
    for lvl, ps in ((0, ps0), (1, ps1)):
        pu = ps.view(np.uint16).reshape(R, -1, 2)
        pslh[lvl] = (np.ascontiguousarray(pu[:, :, 0]),
                     np.ascontiguousarray(pu[:, :, 1]))

    in_maps = []
    for c in range(N_CORES):
        r0 = c * RPC
        lo = c * MPC
        im = {
            "pdgt": pdgt[r0:r0 + RPC],
            "sdrw": sdrw[r0:r0 + RPC],
            "pspw0": pspw[0][r0:r0 + RPC], "pspw1": pspw[1][r0:r0 + RPC],
            "pslo0": pslh[0][0][r0:r0 + RPC], "pshi0": pslh[0][1][r0:r0 + RPC],
            "pslo1": pslh[1][0][r0:r0 + RPC], "pshi1": pslh[1][1][r0:r0 + RPC],
        }
        for lvl in (0, 1):
            idx_pad, emb_pad = hashes[lvl]
            im[f"hi{lvl}"] = np.ascontiguousarray(idx_pad[lo:lo + HSLICE])
            im[f"he{lvl}"] = np.ascontiguousarray(
                emb_pad[lo:lo + HSLICE].reshape(-1))
        im.update(consts)
        in_maps.append(im)
    return in_maps


def kernel(**inputs) -> np.ndarray:
    nc = _get_module()
    in_maps = shard_inputs(inputs)
    res = run_bass_kernel_spmd(nc, in_maps, core_ids=list(range(N_CORES)))
    total = np.float64(0.0)
    for r in res.results:
        total += np.float64(r["out"][0, 0])
    return np.float32(total)
